# revision 2
# baseline (speedup 1.0000x reference)
"""Trainium2 Bass kernel for nn_NodeInference (2-layer GAT + cosine head).

v3 design (SPMD over 8 cores, dst-node sharding):
  Host globally re-assigns nodes to (core, block) bins, balancing per-block
  in-degree.  Self-loops are NOT in the edge lists; each block handles them
  as one extra "virtual chunk" fed from the block's own window rows.

  LAYER 1 HAS NO GATHER: the h1 rows needed per edge slot are a linear
  function of x (host input), so the host ships x[src_e] pre-arranged per
  edge slot (xsTi) and the kernel computes h1aug per chunk with the PE
  (fused dense+edge phase).  This removes the dominant GpSimd dma_gather
  descriptor-generation cost for layer 1 and the whole separate dense
  phase / h1 table of v2.

  P2  edge phase 1 (per dst block of 128 nodes):
      - win rows (own nodes) from xoTi @ W1aug (also a_src/a_dst source)
      - per chunk j: gt[:,j] = xsTi[:,blk,j] @ W1aug  (PE, 2 K-halves)
      - PE broadcast of per-chunk dst-locals -> S_T; a_d per edge = S_T^T @ adwin
      - w_e = exp(min(leakyrelu(a_s + a_d, 0.2), 30))
      - Sw_h[e,d] = (dstf==iota) * w_h (DVE); PSUM accum via one-hot matmuls
        (table rows carry literal 1.0 -> same matmul accumulates sum_w)
      - out1 = leakyrelu(bp_h/SumW_h + b1, 0.01); PE-transpose -> out1T
      - h2aug = out1 @ W2aug -> packed 384-col rows -> cc_in
  P3  AllGather cc_in -> cc_out (row index == global node position)
  P4  edge phase 2 (1 head) over cc_out via dma_gather (content is
      device-computed, so the host trick does not apply)
  P5  head: cos sim vs mu -> outT [8, SHARD_CAP], interleaved into P4
Host scatters per-core outT into the full output via the assignment map.
"""

import sys
from dataclasses import dataclass, field
from contextlib import ExitStack

if "/opt/trn_rl_repo" not in sys.path:
    sys.path.insert(0, "/opt/trn_rl_repo")

import numpy as np

import concourse.bacc as bacc
import concourse.bass as bass
import concourse.mybir as mybir
import concourse.tile as tile
from concourse.bass import AP

P = 128
IN = 256          # input feature dim
H1 = 2            # layer-1 heads
HID = 256         # layer-1 output dim (2*128, concat)
OUT = 256         # layer-2 output dim
KH, MD = 8, 128   # cosine head shape
ROWW = 384        # fp16 cols per packed L2 table row (768B)
HALF = 32768      # int16 table-half split
AF = mybir.ActivationFunctionType
ALU = mybir.AluOpType
DT = mybir.dt


@dataclass
class CFG:
    N: int
    W: int              # world size
    NBLK: int           # dst blocks (128 dsts) per core
    CPB1: int           # chunks per block, layer 1 (no lo/hi split)
    CPL2: int           # lo-half chunks per block, layer 2
    CPH2: int
    idxmaps: object = field(default=None, repr=False)

    @property
    def SHARD_CAP(self):
        return self.NBLK * P

    @property
    def CPB2(self):
        return self.CPL2 + self.CPH2


def build_program(cfg: CFG):
    nc = bacc.Bacc("TRN2", target_bir_lowering=False, debug=False)
    W, NBLK = cfg.W, cfg.NBLK
    AUG1, AUG2 = IN + 4, IN + 2
    f16, bf16, f32 = DT.float16, DT.bfloat16, DT.float32
    i16 = DT.int16
    CPB1, CPB2 = cfg.CPB1, cfg.CPB2

    with tile.TileContext(nc) as tc, ExitStack() as stack:
        dram = stack.enter_context(
            tc.tile_pool(name="dram", bufs=1, space="DRAM"))

        def din(name, shape, dtype):
            return dram.tile(shape, dtype, kind="ExternalInput", name=name,
                             uniquify=False)

        xsTi = din("xsTi", [P, NBLK, CPB1, 2, P], f16)
        xoTi = din("xoTi", [P, NBLK, 2, P], f16)
        w1s = din("w1s", [P, 2, AUG1], f16)
        w2s = din("w2s", [P, 2, AUG2], f16)
        gsd = din("gs", [P, 2, KH * P], f16)
        mus = din("mus", [P, KH * KH], f16)       # block-diag mu^T
        ond = din("onesd", [P, KH * KH], f16)     # block-diag ones
        cmu = din("cmu", [KH, 1], f32)
        b1d = din("b1b", [P, HID], f32)
        b2d = din("b2b", [P, OUT], f32)
        iot = din("iota", [P, P], bf16)
        ioc = din("iotac", [P, 1], f32)           # iota column (partition idx)
        one1 = din("ones1", [1, P], bf16)         # bcast matmul lhsT
        idn = din("ident", [P, P], f32)
        df1 = din("dstf1", [P, NBLK * CPB1], i16)   # bf16 bits: dst-local
        dr1 = din("dstrow1", [1, NBLK * CPB1 * P], bf16)
        is2 = din("isd2", [P, NBLK * CPB2 * 9], i16)
        dr2 = din("dstrow2", [1, NBLK * CPB2 * P], bf16)
        outT = dram.tile([KH, cfg.SHARD_CAP], f32, kind="ExternalOutput",
                         name="outT", uniquify=False)

        cc_in = dram.tile([cfg.SHARD_CAP, ROWW], f16, name="cc_in")
        cc_out = dram.tile([W * cfg.SHARD_CAP, ROWW], f16, name="cc_out",
                           addr_space="Shared" if W > 1 else "Local")

        consts = stack.enter_context(tc.tile_pool(name="consts", bufs=1))
        w1_sb = consts.tile([P, 2, AUG1], f16)
        w2_sb = consts.tile([P, 2, AUG2], f16)
        g_sb = consts.tile([P, 2, KH * P], f16)
        mu_sb = consts.tile([P, KH * KH], f16)
        on_sb = consts.tile([P, KH * KH], f16)
        cmu_sb = consts.tile([KH, 1], f32)
        b1_sb = consts.tile([P, HID], f32)
        b2_sb = consts.tile([P, OUT], f32)
        iota_sb = consts.tile([P, P], bf16)
        iotac_sb = consts.tile([P, 1], f32)
        ones1_sb = consts.tile([1, P], bf16)
        ident_sb = consts.tile([P, P], f32)
        out1T_sb = consts.tile([P, 2, cfg.SHARD_CAP], f16)
        h2fT_sb = consts.tile([P, 2, cfg.SHARD_CAP], f16)

        for dst, src in [(w1_sb, w1s), (w2_sb, w2s), (g_sb, gsd),
                         (mu_sb, mus), (on_sb, ond), (cmu_sb, cmu),
                         (b1_sb, b1d), (b2_sb, b2d), (iota_sb, iot),
                         (iotac_sb, ioc), (ones1_sb, one1),
                         (ident_sb, idn)]:
            nc.sync.dma_start(dst[:], src[:])

        # ================= edge phases ======================================
        def edge_phase(layer, post_block=None):
            estack = ExitStack()
            if layer == 1:
                CPB, nhead = CPB1, 2
                b_sb, out_t, lrelu_out = b1_sb, out1T_sb, True
                px = estack.enter_context(
                    tc.tile_pool(name="p1x", bufs=3))
                pps = estack.enter_context(
                    tc.tile_pool(name="p1ps", bufs=2, space="PSUM"))
            else:
                CPB, nhead = CPB2, 1
                b_sb, out_t, lrelu_out = b2_sb, h2fT_sb, False
                ccrows = W * cfg.SHARD_CAP
                tab_lo = cc_out[0:min(HALF, ccrows), :]
                tab_hi = (cc_out[HALF:ccrows, :] if ccrows > HALF
                          else tab_lo)
                CPL = cfg.CPL2

            pi = estack.enter_context(
                tc.tile_pool(name=f"idx{layer}", bufs=4))
            pg = estack.enter_context(
                tc.tile_pool(name=f"gath{layer}", bufs=3))
            pw = estack.enter_context(
                tc.tile_pool(name=f"win{layer}", bufs=3))
            pb = estack.enter_context(
                tc.tile_pool(name=f"bcps{layer}", bufs=1, space="PSUM"))
            pst = estack.enter_context(tc.tile_pool(name=f"st{layer}", bufs=3))
            pad_ = estack.enter_context(
                tc.tile_pool(name=f"adps{layer}", bufs=1, space="PSUM"))
            pe_ = estack.enter_context(tc.tile_pool(name=f"ew{layer}", bufs=2))
            pm = estack.enter_context(tc.tile_pool(name=f"sw{layer}", bufs=2))
            pp = estack.enter_context(
                tc.tile_pool(name=f"bps{layer}", bufs=2, space="PSUM"))
            pt = estack.enter_context(
                tc.tile_pool(name=f"tps{layer}", bufs=1, space="PSUM"))
            po = estack.enter_context(tc.tile_pool(name=f"epi{layer}", bufs=2))
            ph = estack.enter_context(
                tc.tile_pool(name=f"h2ps{layer}", bufs=1, space="PSUM"))

            NSW = CPB + 1   # chunks + self-loop virtual chunk

            for blk in range(NBLK):
                if layer == 1:
                    # --- fused dense: win (own rows) + gt (per edge slot)
                    xo = px.tile([P, 2, P], f16, tag="xo")
                    nc.sync.dma_start(xo[:], xoTi[:, blk, :, :])
                    xt = px.tile([P, CPB, 2, P], f16, tag="xt")
                    nc.sync.dma_start(xt[:], xsTi[:, blk, :, :, :])
                    dft = pi.tile([P, CPB], i16, tag="dft")
                    nc.sync.dma_start(dft[:], df1[:, blk * CPB:
                                                  (blk + 1) * CPB])
                    dstf = dft[:].bitcast(bf16)
                    drow = pi.tile([1, CPB * P], bf16, tag="drow")
                    nc.sync.dma_start(drow[:], dr1[:, blk * CPB * P:
                                                   (blk + 1) * CPB * P])

                    wps = pps.tile([P, AUG1], f32, tag="ps")
                    for k in range(2):
                        nc.tensor.matmul(wps[:], lhsT=xo[:, k, :],
                                         rhs=w1_sb[:, k, :],
                                         start=(k == 0), stop=(k == 1))
                    # win row layout: [h1 0:128 | 1.0 | h2 129:257 | 1.0]
                    win = pw.tile([P, IN + 2], f16, tag="win")
                    winf4 = pw.tile([P, 4], f32, tag="winf4")
                    nc.scalar.activation(win[:, 0:P], wps[:, 0:P], AF.Copy)
                    nc.scalar.activation(win[:, P + 1:IN + 1],
                                         wps[:, P:IN], AF.Copy)
                    nc.vector.tensor_copy(winf4[:], wps[:, IN:IN + 4])
                    nc.vector.memset(win[:, P:P + 1], 1.0)
                    nc.vector.memset(win[:, IN + 1:IN + 2], 1.0)

                    gt = pg.tile([P, CPB, IN + 2], f16, tag="gt")
                    asv = pg.tile([P, CPB, 2], f32, tag="asv")
                    for j in range(CPB):
                        ps = pps.tile([P, AUG1], f32, tag="ps")
                        for k in range(2):
                            nc.tensor.matmul(ps[:], lhsT=xt[:, j, k, :],
                                             rhs=w1_sb[:, k, :],
                                             start=(k == 0), stop=(k == 1))
                        nc.scalar.activation(gt[:, j, 0:P], ps[:, 0:P],
                                             AF.Copy)
                        nc.scalar.activation(gt[:, j, P + 1:IN + 1],
                                             ps[:, P:IN], AF.Copy)
                        nc.vector.tensor_copy(asv[:, j, :], ps[:, IN:IN + 2])
                    nc.vector.memset(gt[:, :, P:P + 1], 1.0)
                    nc.vector.memset(gt[:, :, IN + 1:IN + 2], 1.0)

                    as_view = asv[:, :, :]
                    as_self = winf4[:, 0:2]
                    ad_self = winf4[:, 2:4]

                    def rhs_chunk(j, c0, c1):
                        return gt[:, j, c0:c1]

                    def rhs_self(c0, c1):
                        return win[:, c0:c1]
                else:
                    # --- per-block inputs (isrc idx + dstf packed in one DMA)
                    cb9 = blk * CPB * 9
                    isd = pi.tile([P, CPB * 9], i16, tag="isd")
                    drow = pi.tile([1, CPB * P], bf16, tag="drow")
                    nc.sync.dma_start(isd[:], is2[:, cb9:cb9 + CPB * 9])
                    nc.sync.dma_start(drow[:], dr2[:, blk * CPB * P:
                                                    (blk + 1) * CPB * P])
                    isrc = isd[:, 0:CPB * 8]
                    dstf = isd[:, CPB * 8:CPB * 9].bitcast(bf16)
                    win = pw.tile([P, ROWW], f16, tag="win")
                    nc.sync.dma_start(win[:],
                                      cc_in[blk * P:(blk + 1) * P, :])
                    winf = win[:].bitcast(f32)

                    # --- src-row gathers (dma_gather caps at 1024 idxs)
                    gt = pg.tile([P, CPB, ROWW], f16, tag="gt")
                    MXC = 8
                    for c0 in range(0, CPL, MXC):
                        c1 = min(c0 + MXC, CPL)
                        nc.gpsimd.dma_gather(
                            gt[:, c0:c1, :], tab_lo,
                            isrc[:, c0 * 8:c1 * 8],
                            (c1 - c0) * P, (c1 - c0) * P, ROWW)
                    for c0 in range(CPL, CPB, MXC):
                        c1 = min(c0 + MXC, CPB)
                        nc.gpsimd.dma_gather(
                            gt[:, c0:c1, :], tab_hi,
                            isrc[:, c0 * 8:c1 * 8],
                            (c1 - c0) * P, (c1 - c0) * P, ROWW)
                    gtf = gt[:].bitcast(f32)

                    as_view = gtf[:, :, 129:130]
                    as_self = winf[:, 129:130]
                    ad_self = winf[:, 130:131]

                    def rhs_chunk(j, c0, c1):
                        return gt[:, j, c0:c1]

                    def rhs_self(c0, c1):
                        return win[:, c0:c1]

                # --- dst-local broadcast (PE) -> dstb (ACT copy) -> S_T via
                # one batched DVE is_equal over all chunks
                dstb = pst.tile([P, CPB * P], bf16, tag="dstb")
                st = pst.tile([P, CPB * P], bf16, tag="st")
                GW = 4  # chunks per broadcast matmul (512 psum cols)
                for g0 in range(0, CPB, GW):
                    g1 = min(g0 + GW, CPB)
                    bc = pb.tile([P, GW * P], f32, tag="bc")
                    nc.tensor.matmul(bc[:, 0:(g1 - g0) * P],
                                     lhsT=ones1_sb[:],
                                     rhs=drow[:, g0 * P:g1 * P],
                                     start=True, stop=True)
                    nc.scalar.activation(dstb[:, g0 * P:g1 * P],
                                         bc[:, 0:(g1 - g0) * P], AF.Copy)
                nc.vector.tensor_scalar(
                    out=st[:], in0=dstb[:], scalar1=iotac_sb[:, 0:1],
                    scalar2=None, op0=ALU.is_equal)

                # --- a_d per edge: ad_ps[e, (j,h)] = S_T_j^T @ adwin
                adw16 = pe_.tile([P, nhead], f16, tag="adw16")
                nc.vector.tensor_copy(adw16[:], ad_self)
                # one accumulation group for the whole tile: start=True zeroes
                # the full psum region, so only the first matmul may start
                # and only the last may stop.
                ad_ps = pad_.tile([P, CPB * nhead], f32, tag="adps")
                for j in range(CPB):
                    nc.tensor.matmul(ad_ps[:, j * nhead:(j + 1) * nhead],
                                     lhsT=st[:, j * P:(j + 1) * P],
                                     rhs=adw16[:], start=(j == 0),
                                     stop=(j == CPB - 1))

                # --- edge weights w = exp(lrelu(a_s + a_d, .2))
                ew = pe_.tile([P, CPB * nhead], f32, tag="ew")
                wv = pe_.tile([P, CPB * nhead], bf16, tag="wv")
                nc.vector.tensor_tensor(
                    ew[:], as_view, ad_ps[:], op=ALU.add)
                ewl = pe_.tile([P, CPB * nhead], f32, tag="ewl")
                nc.vector.tensor_scalar(out=ewl[:], in0=ew[:], scalar1=0.2,
                                        scalar2=None, op0=ALU.mult)
                nc.vector.tensor_tensor(ewl[:], ewl[:], ew[:], op=ALU.max)
                nc.scalar.activation(wv[:], ewl[:], AF.Exp)

                # --- self-loop weights from the window rows
                ws = pe_.tile([P, nhead], f32, tag="ws")
                nc.vector.tensor_tensor(ws[:], as_self, ad_self, op=ALU.add)
                wt = pe_.tile([P, nhead], f32, tag="wt")
                nc.vector.tensor_scalar(out=wt[:], in0=ws[:], scalar1=0.2,
                                        scalar2=None, op0=ALU.mult)
                nc.vector.tensor_tensor(wt[:], wt[:], ws[:], op=ALU.max)
                nc.scalar.activation(wt[:], wt[:], AF.Exp)

                # --- batched one-hot scatter matrices:
                #     s_all[e,(j,d)] = (dstf[e,j] == iota[d])
                #     sw_h[e,(j,d)]  = s_all * w_h[e,j]   (stride-0 bcasts)
                sall = pm.tile([P, CPB * P], bf16, tag="sall")
                dfa = dstf[:, 0:1]
                dview = AP(dfa.tensor, dfa.offset,
                           [dfa.ap[0], [1, CPB], [0, P]])
                ioa = iota_sb[:]
                iview = AP(ioa.tensor, ioa.offset,
                           [ioa.ap[0], [0, CPB], [1, P]])
                nc.vector.tensor_tensor(sall[:], dview, iview,
                                        op=ALU.is_equal)
                swh = []
                for h in range(nhead):
                    wvh = wv[:, h:h + 1]
                    wview = AP(wvh.tensor, wvh.offset,
                               [wvh.ap[0], [nhead, CPB], [0, P]])
                    swt = pm.tile([P, CPB * P], bf16, tag=f"swh{h}")
                    nc.vector.tensor_tensor(swt[:], sall[:], wview,
                                            op=ALU.mult)
                    swh.append(swt)

                # --- weighted one-hot scatter accumulation
                # single psum accumulation group across all j/h matmuls:
                # start only on the first matmul, stop only on the very last.
                # The rows carry a literal 1.0 after each head's features, so
                # one matmul accumulates both the weighted feature sum and the
                # softmax denominator:
                # bp layout [f1 0:128 | sumw1 | f2 129:257 | sumw2] (2 heads)
                # or [f 0:256 | sumw] (1 head).
                HB = P + 1 if nhead == 2 else IN + 1
                bp = pp.tile([P, nhead * HB], f32, tag="bp")
                for j in range(NSW):
                    last = (j == NSW - 1)
                    selfc = (j == CPB)
                    for h in range(nhead):
                        if selfc:
                            sw = pm.tile([P, P], bf16, tag="sw")
                            nc.vector.tensor_scalar(
                                out=sw[:], in0=iota_sb[:],
                                scalar1=iotac_sb[:, 0:1],
                                scalar2=wt[:, h:h + 1],
                                op0=ALU.is_equal, op1=ALU.mult)
                            lhsT = sw[:]
                        else:
                            lhsT = swh[h][:, j * P:(j + 1) * P]
                        c0, c1 = h * HB, (h + 1) * HB
                        rhs = (rhs_self(c0, c1) if selfc
                               else rhs_chunk(j, c0, c1))
                        nc.tensor.matmul(bp[:, c0:c1], lhsT=lhsT,
                                         rhs=rhs,
                                         start=(j == 0 and h == 0),
                                         stop=(last and h == nhead - 1))

                # ---- block epilogue
                rec = po.tile([P, nhead], f32, tag="rec")
                for h in range(nhead):
                    nc.vector.reciprocal(rec[:, h:h + 1],
                                         bp[:, (h + 1) * HB - 1:
                                            (h + 1) * HB])
                ti = po.tile([P, IN], f32, tag="ti")
                if nhead == 2:
                    nc.scalar.activation(ti[:, 0:P], bp[:, 0:P], AF.Copy,
                                         scale=rec[:, 0:1])
                    nc.scalar.activation(ti[:, P:IN], bp[:, HB:HB + P],
                                         AF.Copy, scale=rec[:, 1:2])
                else:
                    nc.scalar.activation(ti[:], bp[:, 0:IN], AF.Copy,
                                         scale=rec[:, 0:1])
                nc.vector.tensor_tensor(ti[:], ti[:], b_sb[:], op=ALU.add)
                if lrelu_out:
                    # lrelu(x) = 0.01x + relu(0.99x)
                    tr = po.tile([P, IN], f32, tag="tr")
                    nc.scalar.activation(tr[:], ti[:], AF.Relu, scale=0.99)
                    nc.vector.scalar_tensor_tensor(
                        out=ti[:], in0=ti[:], scalar=0.01, in1=tr[:],
                        op0=ALU.mult, op1=ALU.add)
                for k in range(2):
                    tp = pt.tile([P, P], f32, tag="tp")
                    nc.tensor.transpose(tp[:], ti[:, k * P:(k + 1) * P],
                                        ident_sb[:])
                    nc.scalar.activation(out_t[:, k, blk * P:(blk + 1) * P],
                                         tp[:], AF.Copy)

                if layer == 1:
                    hp = ph.tile([P, AUG2], f32, tag="hp")
                    for k in range(2):
                        nc.tensor.matmul(
                            hp[:],
                            lhsT=out1T_sb[:, k, blk * P:(blk + 1) * P],
                            rhs=w2_sb[:, k, :], start=(k == 0), stop=(k == 1))
                    # L2 row: [h 0:256 | 1.0 | a_s f32 @129 | a_d f32 @130]
                    row2 = po.tile([P, ROWW], f16, tag="row2")
                    nc.scalar.activation(row2[:, 0:OUT], hp[:, 0:OUT], AF.Copy)
                    nc.vector.memset(row2[:, OUT:OUT + 2], 1.0)
                    r2f = row2[:].bitcast(f32)
                    nc.vector.tensor_copy(r2f[:, 129:131],
                                          hp[:, OUT:OUT + 2])
                    nc.sync.dma_start(
                        cc_in[blk * P:(blk + 1) * P, 0:OUT + 6],
                        row2[:, 0:OUT + 6])

                if post_block is not None:
                    post_block(blk)

            estack.close()

        edge_phase(1)

        nc.gpsimd.collective_compute(
            "AllGather", ALU.bypass,
            replica_groups=[list(range(W))],
            ins=[cc_in[:]], outs=[cc_out[:]])

        # ====== P5 head, interleaved into edge phase 2 ======================
        hstack = ExitStack()
        hps = hstack.enter_context(
            tc.tile_pool(name="hps", bufs=1, space="PSUM"))
        hsb = hstack.enter_context(tc.tile_pool(name="hsb", bufs=2))
        sps = hstack.enter_context(
            tc.tile_pool(name="sps", bufs=1, space="PSUM"))
        hepi = hstack.enter_context(tc.tile_pool(name="hepi", bufs=2))

        NTL = []
        _st = 0
        while _st < cfg.SHARD_CAP:
            _w = min(512, cfg.SHARD_CAP - _st)
            NTL.append((_st, _w))
            _st += _w

        def head_slice(st, wdt):
            nump = sps.tile([KH, 512], f32, tag="nump")
            nrmp = sps.tile([KH, 512], f32, tag="nrmp")
            for k in range(KH):
                hp = hps.tile([P, 512], f32, tag="hp")
                for f in range(2):
                    nc.tensor.matmul(hp[:, 0:wdt],
                                     lhsT=g_sb[:, f, k * P:(k + 1) * P],
                                     rhs=h2fT_sb[:, f, st:st + wdt],
                                     start=(f == 0), stop=(f == 1))
                h16 = hsb.tile([P, 512], f16, tag="h16")
                sq16 = hsb.tile([P, 512], f16, tag="sq16")
                nc.vector.tensor_copy(h16[:, 0:wdt], hp[:, 0:wdt])
                nc.scalar.activation(sq16[:, 0:wdt], hp[:, 0:wdt], AF.Square)
                nc.tensor.matmul(nump[:, 0:wdt],
                                 lhsT=mu_sb[:, k * KH:(k + 1) * KH],
                                 rhs=h16[:, 0:wdt], start=(k == 0),
                                 stop=(k == KH - 1))
                nc.tensor.matmul(nrmp[:, 0:wdt],
                                 lhsT=on_sb[:, k * KH:(k + 1) * KH],
                                 rhs=sq16[:, 0:wdt], start=(k == 0),
                                 stop=(k == KH - 1))
            sq = hepi.tile([KH, 512], f32, tag="sqr")
            # sqrt(x) = exp(0.5*ln(x)) -- keeps ACT on the ln/exp table set
            nc.scalar.activation(sq[:, 0:wdt], nrmp[:, 0:wdt], AF.Ln)
            nc.scalar.activation(sq[:, 0:wdt], sq[:, 0:wdt], AF.Exp,
                                 scale=0.5)
            nc.vector.tensor_scalar(out=sq[:, 0:wdt], in0=sq[:, 0:wdt],
                                    scalar1=cmu_sb[:, 0:1], scalar2=1e-8,
                                    op0=ALU.mult, op1=ALU.max)
            nc.vector.reciprocal(sq[:, 0:wdt], sq[:, 0:wdt])
            res = hepi.tile([KH, 512], f32, tag="res")
            nc.vector.tensor_tensor(res[:, 0:wdt], nump[:, 0:wdt],
                                    sq[:, 0:wdt], op=ALU.mult)
            nc.sync.dma_start(outT[:, st:st + wdt], res[:, 0:wdt])

        _emitted = [0]

        def _post(blk):
            done = (blk + 1) * P
            while _emitted[0] < len(NTL):
                st, wdt = NTL[_emitted[0]]
                if st + wdt > done:
                    break
                head_slice(st, wdt)
                _emitted[0] += 1

        edge_phase(2, post_block=_post)
        while _emitted[0] < len(NTL):
            st, wdt = NTL[_emitted[0]]
            head_slice(st, wdt)
            _emitted[0] += 1
        hstack.close()

    nc.compile()
    return nc


# ======================= host-side preparation ==============================

def _wrap16(flat):
    """idx flat [n] -> wrapped int16 [128, n//16]; pos i -> (i%16, i//16),
    replicated across the 8 Q7-core stripes."""
    n = len(flat)
    out = np.zeros((P, n // 16), np.int16)
    cols = np.arange(n) // 16
    rows = np.arange(n) % 16
    for r in range(8):
        out[r * 16 + rows, cols] = flat
    return out


def _balance_bins(deg, nbins, cap):
    """Greedy multiway partition: assign nodes to bins balancing total degree,
    each bin holding at most `cap` nodes.  Returns bin id per node."""
    import heapq
    n = len(deg)
    order = np.argsort(-deg, kind="stable")
    binid = np.empty(n, np.int32)
    counts = np.zeros(nbins, np.int32)
    heap = [(0, b) for b in range(nbins)]
    heapq.heapify(heap)
    for nd in order:
        while True:
            load, b = heapq.heappop(heap)
            if counts[b] < cap:
                break
        binid[nd] = b
        counts[b] += 1
        if counts[b] < cap:
            heapq.heappush(heap, (load + int(deg[nd]), b))
    return binid


def prep_host(x, edge_index, W1, a_src1, a_dst1, b1, W2, a_src2, a_dst2, b2,
              g, mu, world=8):
    import ml_dtypes
    x16 = np.asarray(x, np.float32).astype(np.float16)
    N = x16.shape[0]
    NBLK = int(np.ceil(N / world / P))
    CAP = NBLK * P
    nbins = world * NBLK

    src = np.asarray(edge_index[0]).astype(np.int64)
    dst = np.asarray(edge_index[1]).astype(np.int64)

    # --- balanced global node -> (core, block, slot) assignment
    deg = np.bincount(dst, minlength=N)
    binid = _balance_bins(deg, nbins, P)
    # slot order within a bin: ascending node id
    order = np.lexsort((np.arange(N), binid))
    gpos = np.empty(N, np.int64)          # node -> global table position
    bin_start = np.arange(nbins, dtype=np.int64) * P
    nxt = bin_start.copy()
    for nd in order:
        b = binid[nd]
        gpos[nd] = nxt[b]
        nxt[b] += 1
    node_core = binid // NBLK
    node_blk = binid % NBLK

    # per-core list of node ids in shard slot order (-1 = empty slot)
    idxmaps = []
    for c in range(world):
        m = np.full(CAP, -1, np.int64)
        mask = node_core == c
        local = gpos[mask] - c * CAP
        m[local] = np.nonzero(mask)[0]
        idxmaps.append(m)

    # --- edges grouped by (core, block) of dst
    ecore = node_core[dst]
    eblk = node_blk[dst]
    gkey = ecore * NBLK + eblk
    gorder = np.argsort(gkey, kind="stable")
    srcg, dstg, gkeyg = src[gorder], dst[gorder], gkey[gorder]
    starts = np.concatenate(
        [[0], np.cumsum(np.bincount(gkeyg, minlength=nbins))])

    ed = {}
    CPB1 = CPL2 = CPH2 = 1
    for c in range(world):
        for b in range(NBLK):
            gid = c * NBLK + b
            es = srcg[starts[gid]:starts[gid + 1]]
            eds = dstg[starts[gid]:starts[gid + 1]]
            dloc = (gpos[eds] - c * CAP - b * P).astype(np.int64)
            l2 = gpos[es]                  # layer-2 table row (= global pos)
            lo2 = l2 < HALF
            ed[(c, b)] = (es, l2, lo2, dloc)
            CPB1 = max(CPB1, int(np.ceil(len(es) / P)))
            CPL2 = max(CPL2, int(np.ceil(lo2.sum() / P)))
            CPH2 = max(CPH2, int(np.ceil((~lo2).sum() / P)))

    cfg = CFG(N=N, W=world, NBLK=NBLK, CPB1=CPB1,
              CPL2=CPL2, CPH2=CPH2, idxmaps=idxmaps)
    CPB2 = cfg.CPB2

    def build_l1(c):
        """xsTi [P,NBLK,CPB1,2,P] f16, dstf1 [P,NBLK*CPB1] bf16->int16,
        dr1 [1, NBLK*CPB1*P] bf16."""
        srcs = np.zeros((NBLK, CPB1 * P), np.int64)
        dls = np.full((NBLK, CPB1 * P), -1, np.int64)
        for b in range(NBLK):
            es, _, _, dloc = ed[(c, b)]
            srcs[b, :len(es)] = es
            dls[b, :len(es)] = dloc
        xs = x16[srcs.ravel()]                       # [NBLK*CPB1*P, IN]
        xsT = np.ascontiguousarray(
            xs.reshape(NBLK, CPB1, P, 2, P).transpose(4, 0, 1, 3, 2))
        dstf = np.ascontiguousarray(
            dls.reshape(NBLK, CPB1, P).transpose(2, 0, 1)
        ).reshape(P, NBLK * CPB1).astype(np.float32)
        dstf = dstf.astype(ml_dtypes.bfloat16).view(np.int16)
        dr = dls.reshape(1, NBLK * CPB1 * P).astype(np.float32)
        return xsT, dstf, dr.astype(ml_dtypes.bfloat16)

    def build_l2(c):
        isd = np.zeros((P, NBLK * CPB2 * 9), np.int16)
        drow = np.full((1, NBLK * CPB2 * P), -1.0, np.float32)
        for b in range(NBLK):
            _, l2, lo2, dloc = ed[(c, b)]
            fl = np.zeros(CPB2 * P, np.int64)     # slot -> table idx (pad 0)
            fd = np.full(CPB2 * P, -1, np.int64)  # slot -> dst_local (pad -1)
            ilo = np.where(lo2)[0]
            ihi = np.where(~lo2)[0]
            fl[:len(ilo)] = l2[ilo]
            fd[:len(ilo)] = dloc[ilo]
            fl[CPL2 * P:CPL2 * P + len(ihi)] = l2[ihi] - HALF
            fd[CPL2 * P:CPL2 * P + len(ihi)] = dloc[ihi]
            cb9 = b * CPB2 * 9
            isd[:, cb9:cb9 + CPB2 * 8] = _wrap16(fl)
            dloc_t = fd.reshape(CPB2, P).T.astype(np.float32)
            isd[:, cb9 + CPB2 * 8:cb9 + CPB2 * 9] = \
                dloc_t.astype(ml_dtypes.bfloat16).view(np.int16)
            drow[0, b * CPB2 * P:(b + 1) * CPB2 * P] = fd.astype(np.float32)
        return isd, drow.astype(ml_dtypes.bfloat16)

    # weights
    W1 = np.asarray(W1, np.float32)
    W2 = np.asarray(W2, np.float32)
    W1r = W1.reshape(H1, MD, IN)
    Ps1 = np.einsum("hdi,hd->ih", W1r, np.asarray(a_src1, np.float32))
    Pd1 = np.einsum("hdi,hd->ih", W1r, np.asarray(a_dst1, np.float32))
    W1aug = np.concatenate([W1.T, Ps1, Pd1], axis=1)
    Ps2 = W2.T @ np.asarray(a_src2, np.float32)[0][:, None]
    Pd2 = W2.T @ np.asarray(a_dst2, np.float32)[0][:, None]
    W2aug = np.concatenate([W2.T, Ps2, Pd2], axis=1)
    AUG1, AUG2 = IN + 4, IN + 2
    w1s = W1aug.reshape(2, P, AUG1).transpose(1, 0, 2).astype(np.float16)
    w2s = W2aug.reshape(2, P, AUG2).transpose(1, 0, 2).astype(np.float16)

    gm = np.asarray(g, np.float32)
    gsd = gm.reshape(2, P, KH * P).transpose(1, 0, 2).astype(np.float16)
    mu = np.asarray(mu, np.float32)
    mus = np.zeros((P, KH * KH), np.float16)
    onesd = np.zeros((P, KH * KH), np.float16)
    for k in range(KH):
        mus[:, k * KH + k] = mu[k, :]
        onesd[:, k * KH + k] = 1.0
    cmu = np.linalg.norm(mu, axis=1)[:, None].astype(np.float32)
    b1b = np.broadcast_to(np.asarray(b1, np.float32), (P, HID)).copy()
    b2b = np.broadcast_to(np.asarray(b2, np.float32), (P, OUT)).copy()
    iota = np.broadcast_to(np.arange(P, dtype=np.float32),
                           (P, P)).astype(ml_dtypes.bfloat16)
    iotac = np.arange(P, dtype=np.float32)[:, None]
    ones1 = np.ones((1, P), ml_dtypes.bfloat16)
    ident = np.eye(P, dtype=np.float32)

    shared = dict(w1s=w1s, w2s=w2s, gs=gsd, mus=mus, onesd=onesd, cmu=cmu,
                  b1b=b1b, b2b=b2b, iota=iota, iotac=iotac, ones1=ones1,
                  ident=ident)
    in_maps = []
    for c in range(world):
        m = idxmaps[c]
        own = np.where(m >= 0, m, 0)
        xo = x16[own]
        xo[m < 0] = 0
        xoT = np.ascontiguousarray(
            xo.reshape(NBLK, P, 2, P).transpose(3, 0, 2, 1))
        xsT, dstf, dr1 = build_l1(c)
        i2, r2 = build_l2(c)
        mm = dict(shared)
        mm.update(xsTi=xsT, xoTi=xoT, dstf1=dstf, dstrow1=dr1,
                  isd2=i2, dstrow2=r2)
        in_maps.append(mm)
    return cfg, in_maps


def assemble(cfg, outs):
    N = cfg.N
    full = np.zeros((N, KH), np.float32)
    for c in range(cfg.W):
        o = np.asarray(outs[c]["outT"])      # [KH, SHARD_CAP]
        m = cfg.idxmaps[c]
        valid = m >= 0
        full[m[valid], :] = o[:, valid].T
    return full


_CACHE = {}


def kernel(**inputs):
    world = 8
    cfg, in_maps = prep_host(world=world, **inputs)
    key = (cfg.N, cfg.W, cfg.CPB1, cfg.CPB2)
    if key not in _CACHE:
        _CACHE[key] = build_program(cfg)
    nc = _CACHE[key]

    from concourse.bass_utils import run_bass_kernel_spmd
    res = run_bass_kernel_spmd(nc, in_maps, core_ids=list(range(world)))
    return assemble(cfg, res.results)


# revision 5
# speedup vs baseline: 1.2803x; 1.2803x over previous
"""Trainium2 Bass kernel for nn_NodeInference (2-layer GAT + cosine head).

v4 design (SPMD over 8 cores, dst-node sharding, hybrid gather/dense):
  Host globally re-assigns nodes to (core, block) bins, balancing per-block
  in-degree.  Both GAT layers share ONE edge-slot layout (chunks of 128
  edges per dst block, split lo/hi by global table row for int16 gather
  indices), so the per-edge one-hot matrices are built once on the host and
  shipped:
     std  [e-transposed]  st[d,(j,e)]  = (dloc[j,e]==d)   (ad lookup lhsT)
     sald                 sall[e,(j,d)] = (dloc[j,e]==d)  (scatter base)
  This removes the per-block PE broadcast + DVE is_equal chains of v2.

  P1  sharded dense: each core computes h1aug only for its OWN 6272 nodes
      -> cc1_in rows [h1|1|h2|1|as f32 x2|ad f32 x2] (768B)
  AG1 AllGather cc1_in -> cc1_out (global h1 table), in 2 pieces
  P2  edge phase 1 per dst block: chunks are HYBRID:
      - gather chunks: dma_gather rows from cc1_out (GpSimd)
      - dense chunks:  gt[:,j] = x[src_e] @ W1aug on the PE (x[src_e] is a
        host input, shipped pre-arranged per edge slot in xsTi) -- trades
        GpSimd descriptor-generation time for PE time to balance engines
      - a_d per edge = st_j^T @ adwin;  w_e = exp(min(lrelu(a_s+a_d,.2),30))
      - scatter: bp += (sall*w_h)_j^T @ rows_j  (rows carry literal 1.0 so
        the same matmul accumulates the softmax denominator)
      - epilogue -> out1T; h2aug rows -> cc2_in
      Blocks are software-pipelined (stage A: dma/dense/gather/ad of block
      b+1 emitted before stage B: ew/swh/scatter/epilogue of block b) to
      avoid in-order PE stalls.
  AG2 AllGather cc2_in -> cc2_out in 2 pieces, piece 0 issued mid-phase
  P4  edge phase 2: all chunks gathered from cc2_out (content is
      device-computed, so the host x-trick cannot apply)
  P5  head: cos sim vs mu -> outT [8, SHARD_CAP], interleaved into P4
Host scatters per-core outT into the full output via the assignment map.
"""

import sys
from dataclasses import dataclass, field
from contextlib import ExitStack

if "/opt/trn_rl_repo" not in sys.path:
    sys.path.insert(0, "/opt/trn_rl_repo")

import numpy as np

import concourse.bacc as bacc
import concourse.bass as bass
import concourse.mybir as mybir
import concourse.tile as tile
from concourse.bass import AP

P = 128
IN = 256          # input feature dim
H1 = 2            # layer-1 heads
HID = 256         # layer-1 output dim (2*128, concat)
OUT = 256         # layer-2 output dim
KH, MD = 8, 128   # cosine head shape
ROWW = 384        # fp16 cols per packed table row (768B)
HALF = 32768      # int16 table-half split
DENSE_LO = 3      # layer-1 lo chunks computed on the PE instead of gathered
DENSE_HI = 2      # layer-1 hi chunks computed on the PE
AF = mybir.ActivationFunctionType
ALU = mybir.AluOpType
DT = mybir.dt


@dataclass
class CFG:
    N: int
    W: int              # world size
    NBLK: int           # dst blocks (128 dsts) per core
    CPL: int            # lo-half chunks per block
    CPH: int
    idxmaps: object = field(default=None, repr=False)

    @property
    def SHARD_CAP(self):
        return self.NBLK * P

    @property
    def CPB(self):
        return self.CPL + self.CPH

    @property
    def DCL(self):
        return min(DENSE_LO, self.CPL)

    @property
    def DCH(self):
        return min(DENSE_HI, self.CPH)

    @property
    def DCT(self):
        return self.DCL + self.DCH

    @property
    def PB0(self):       # blocks in AllGather piece 0
        return (self.NBLK + 1) // 2


def build_program(cfg: CFG):
    nc = bacc.Bacc("TRN2", target_bir_lowering=False, debug=False)
    W, NBLK = cfg.W, cfg.NBLK
    AUG1, AUG2 = IN + 4, IN + 2
    f16, bf16, f32 = DT.float16, DT.bfloat16, DT.float32
    i16 = DT.int16
    CPB, CPL, CPH = cfg.CPB, cfg.CPL, cfg.CPH
    DCL, DCH = cfg.DCL, cfg.DCH
    PB0, PB1 = cfg.PB0, cfg.NBLK - cfg.PB0
    CAP = cfg.SHARD_CAP

    with tile.TileContext(nc) as tc, ExitStack() as stack:
        dram = stack.enter_context(
            tc.tile_pool(name="dram", bufs=1, space="DRAM"))

        def din(name, shape, dtype):
            return dram.tile(shape, dtype, kind="ExternalInput", name=name,
                             uniquify=False)

        xoTi = din("xoTi", [P, NBLK, 2, P], f16)
        xsTi = din("xsTi", [P, NBLK, cfg.DCT, 2, P], f16)
        isd = din("isd", [P, NBLK * CPB * 8], i16)
        std = din("std", [P, NBLK * CPB * P], bf16)
        sald = din("sald", [P, NBLK * CPB * P], bf16)
        w1s = din("w1s", [P, 2, AUG1], f16)
        w2s = din("w2s", [P, 2, AUG2], f16)
        gsd = din("gs", [P, 2, KH * P], f16)
        mus = din("mus", [P, KH * KH], f16)       # block-diag mu^T
        ond = din("onesd", [P, KH * KH], f16)     # block-diag ones
        cmu = din("cmu", [KH, 1], f32)
        b1d = din("b1b", [P, HID], f32)
        b2d = din("b2b", [P, OUT], f32)
        idn = din("ident", [P, P], f32)
        idb = din("identb", [P, P], bf16)
        outT = dram.tile([KH, CAP], f32, kind="ExternalOutput",
                         name="outT", uniquify=False)

        shsp = "Shared" if W > 1 else "Local"
        cc1_in = dram.tile([CAP, ROWW], f16, name="cc1_in")
        cc1_p0 = dram.tile([W * PB0 * P, ROWW], f16, name="cc1_p0",
                           addr_space=shsp)
        cc1_p1 = dram.tile([W * PB1 * P, ROWW], f16, name="cc1_p1",
                           addr_space=shsp)
        cc2_in = dram.tile([CAP, ROWW], f16, name="cc2_in")
        cc2_p0 = dram.tile([W * PB0 * P, ROWW], f16, name="cc2_p0",
                           addr_space=shsp)
        cc2_p1 = dram.tile([W * PB1 * P, ROWW], f16, name="cc2_p1",
                           addr_space=shsp)

        consts = stack.enter_context(tc.tile_pool(name="consts", bufs=1))
        w1_sb = consts.tile([P, 2, AUG1], f16)
        w2_sb = consts.tile([P, 2, AUG2], f16)
        g_sb = consts.tile([P, 2, KH * P], f16)
        mu_sb = consts.tile([P, KH * KH], f16)
        on_sb = consts.tile([P, KH * KH], f16)
        cmu_sb = consts.tile([KH, 1], f32)
        b1_sb = consts.tile([P, HID], f32)
        b2_sb = consts.tile([P, OUT], f32)
        ident_sb = consts.tile([P, P], f32)
        identb_sb = consts.tile([P, P], bf16)
        out1T_sb = consts.tile([P, 2, CAP], f16)
        h2fT_sb = consts.tile([P, 2, CAP], f16)

        for dst, src in [(w1_sb, w1s), (w2_sb, w2s), (g_sb, gsd),
                         (mu_sb, mus), (on_sb, ond), (cmu_sb, cmu),
                         (b1_sb, b1d), (b2_sb, b2d),
                         (ident_sb, idn), (identb_sb, idb)]:
            nc.sync.dma_start(dst[:], src[:])

        def ag_piece(cin, cout_p, pc):
            """AllGather piece pc (0/1) of cin into its own Shared tensor."""
            lr = (0, PB0 * P) if pc == 0 else (PB0 * P, CAP)
            nc.gpsimd.collective_compute(
                "AllGather", ALU.bypass,
                replica_groups=[list(range(W))],
                ins=[cin[lr[0]:lr[1], :]], outs=[cout_p[:, :]])

        # ================= P1: sharded dense (own nodes only) ===============
        with tc.tile_pool(name="p1x", bufs=3) as p1x, \
             tc.tile_pool(name="p1ps", bufs=3, space="PSUM") as p1ps, \
             tc.tile_pool(name="p1row", bufs=3) as p1row:
            for blk in range(NBLK):
                xo = p1x.tile([P, 2, P], f16, tag="xo")
                nc.sync.dma_start(xo[:], xoTi[:, blk, :, :])
                ps = p1ps.tile([P, AUG1], f32, tag="ps")
                for k in range(2):
                    nc.tensor.matmul(ps[:], lhsT=xo[:, k, :],
                                     rhs=w1_sb[:, k, :],
                                     start=(k == 0), stop=(k == 1))
                # row: [h1 0:128 | 1.0 | h2 129:257 | 1.0 | as f32 x2 | ad..]
                row = p1row.tile([P, 272], f16, tag="row")
                rf32 = row[:].bitcast(f32)
                nc.scalar.activation(row[:, 0:P], ps[:, 0:P], AF.Copy)
                nc.scalar.activation(row[:, P + 1:IN + 1],
                                     ps[:, P:IN], AF.Copy)
                nc.vector.tensor_copy(rf32[:, 129:133], ps[:, IN:IN + 4])
                nc.vector.memset(row[:, P:P + 1], 1.0)
                nc.vector.memset(row[:, IN + 1:IN + 2], 1.0)
                nc.sync.dma_start(
                    cc1_in[blk * P:(blk + 1) * P, 0:IN + 10],
                    row[:, 0:IN + 10])
                if blk == PB0 - 1:
                    ag_piece(cc1_in, cc1_p0, 0)
            ag_piece(cc1_in, cc1_p1, 1)

        # ================= edge phases ======================================
        def edge_phase(layer, post_block=None):
            if layer == 1:
                nhead = 2
                b_sb, out_t, lrelu_out = b1_sb, out1T_sb, True
                as_off, ad_off = 129, 131   # f32 col offsets in table rows
                cin, tab_lo, tab_hi = cc1_in, cc1_p0[:, :], cc1_p1[:, :]
                # chunk roles: lo [0:CPL-DCL] gather, [CPL-DCL:CPL] dense
                #              hi [CPL:CPB-DCH] gather, [CPB-DCH:CPB] dense
                g_rngs = [(0, CPL - DCL), (CPL, CPB - DCH)]
                d_rngs = [(CPL - DCL, CPL, 0), (CPB - DCH, CPB, DCL)]
            else:
                nhead = 1
                b_sb, out_t, lrelu_out = b2_sb, h2fT_sb, False
                as_off, ad_off = 129, 130
                cin, tab_lo, tab_hi = cc2_in, cc2_p0[:, :], cc2_p1[:, :]
                g_rngs = [(0, CPL), (CPL, CPB)]
                d_rngs = []

            estack = ExitStack()
            pi = estack.enter_context(
                tc.tile_pool(name=f"idx{layer}", bufs=3))
            pso = estack.enter_context(
                tc.tile_pool(name=f"soh{layer}", bufs=3))
            pg = estack.enter_context(
                tc.tile_pool(name=f"gath{layer}", bufs=3))
            pw = estack.enter_context(
                tc.tile_pool(name=f"win{layer}", bufs=3))
            px = (estack.enter_context(tc.tile_pool(name="dsx", bufs=3))
                  if d_rngs else None)
            pps = (estack.enter_context(
                tc.tile_pool(name="dsps", bufs=2, space="PSUM"))
                if d_rngs else None)
            pad_ = estack.enter_context(
                tc.tile_pool(name=f"adps{layer}", bufs=2, space="PSUM"))
            pe_ = estack.enter_context(tc.tile_pool(name=f"ew{layer}", bufs=3))
            pm = estack.enter_context(tc.tile_pool(name=f"sw{layer}", bufs=2))
            pp = estack.enter_context(
                tc.tile_pool(name=f"bps{layer}", bufs=2, space="PSUM"))
            pt = estack.enter_context(
                tc.tile_pool(name=f"tps{layer}", bufs=1, space="PSUM"))
            po = estack.enter_context(tc.tile_pool(name=f"epi{layer}", bufs=2))
            ph = estack.enter_context(
                tc.tile_pool(name=f"h2ps{layer}", bufs=1, space="PSUM"))

            NSW = CPB + 1   # chunks + self-loop virtual chunk
            state = {}

            def stage_a(blk):
                cb8 = blk * CPB * 8
                isdt = pi.tile([P, CPB * 8], i16, tag="isd")
                nc.sync.dma_start(isdt[:], isd[:, cb8:cb8 + CPB * 8])
                st = pso.tile([P, CPB * P], bf16, tag="st")
                nc.sync.dma_start(st[:], std[:, blk * CPB * P:
                                             (blk + 1) * CPB * P])
                sall = pso.tile([P, CPB * P], bf16, tag="sall")
                nc.sync.dma_start(sall[:], sald[:, blk * CPB * P:
                                                (blk + 1) * CPB * P])
                win = pw.tile([P, ROWW], f16, tag="win")
                nc.sync.dma_start(win[:], cin[blk * P:(blk + 1) * P, :])
                winf = win[:].bitcast(f32)

                gt = pg.tile([P, CPB, ROWW], f16, tag="gt")
                gtf = gt[:].bitcast(f32)
                # gather chunks
                MXC = 8
                for r0, r1 in g_rngs:
                    for c0 in range(r0, r1, MXC):
                        c1 = min(c0 + MXC, r1)
                        nc.gpsimd.dma_gather(
                            gt[:, c0:c1, :],
                            tab_lo if r0 == 0 else tab_hi,
                            isdt[:, c0 * 8:c1 * 8],
                            (c1 - c0) * P, (c1 - c0) * P, ROWW)
                # dense chunks (layer 1 only)
                if d_rngs:
                    xt = px.tile([P, cfg.DCT, 2, P], f16, tag="xt")
                    nc.sync.dma_start(xt[:], xsTi[:, blk, :, :, :])
                    for r0, r1, xb in d_rngs:
                        for j in range(r0, r1):
                            ps = pps.tile([P, AUG1], f32, tag="ps")
                            xi = xb + (j - r0)
                            for k in range(2):
                                nc.tensor.matmul(ps[:], lhsT=xt[:, xi, k, :],
                                                 rhs=w1_sb[:, k, :],
                                                 start=(k == 0), stop=(k == 1))
                            nc.scalar.activation(gt[:, j, 0:P], ps[:, 0:P],
                                                 AF.Copy)
                            nc.scalar.activation(gt[:, j, P + 1:IN + 1],
                                                 ps[:, P:IN], AF.Copy)
                            nc.vector.tensor_copy(gtf[:, j, 129:131],
                                                  ps[:, IN:IN + 2])
                        nc.vector.memset(gt[:, r0:r1, P:P + 1], 1.0)
                        nc.vector.memset(gt[:, r0:r1, IN + 1:IN + 2], 1.0)

                # a_d per edge: ad_ps[e, (j,h)] = st_j^T @ adwin
                adw16 = pe_.tile([P, nhead], f16, tag="adw16")
                nc.vector.tensor_copy(adw16[:],
                                      winf[:, ad_off:ad_off + nhead])
                ad_ps = pad_.tile([P, CPB * nhead], f32, tag="adps")
                for j in range(CPB):
                    nc.tensor.matmul(ad_ps[:, j * nhead:(j + 1) * nhead],
                                     lhsT=st[:, j * P:(j + 1) * P],
                                     rhs=adw16[:], start=(j == 0),
                                     stop=(j == CPB - 1))
                state[blk] = (gt, gtf, win, winf, sall, ad_ps)

            def stage_b(blk):
                gt, gtf, win, winf, sall, ad_ps = state.pop(blk)
                # edge weights w = exp(lrelu(a_s + a_d, .2))
                ew = pe_.tile([P, CPB * nhead], f32, tag="ew")
                wv = pe_.tile([P, CPB * nhead], bf16, tag="wv")
                as_v = gtf[:, :, as_off:as_off + nhead]
                nc.vector.tensor_tensor(ew[:], as_v, ad_ps[:], op=ALU.add)
                ewl = pe_.tile([P, CPB * nhead], f32, tag="ewl")
                nc.vector.tensor_scalar(out=ewl[:], in0=ew[:], scalar1=0.2,
                                        scalar2=None, op0=ALU.mult)
                nc.vector.tensor_tensor(ewl[:], ewl[:], ew[:], op=ALU.max)
                nc.scalar.activation(wv[:], ewl[:], AF.Exp)

                # self-loop weights from the window rows
                ws = pe_.tile([P, nhead], f32, tag="ws")
                nc.vector.tensor_tensor(
                    ws[:], winf[:, as_off:as_off + nhead],
                    winf[:, ad_off:ad_off + nhead], op=ALU.add)
                wt = pe_.tile([P, nhead], f32, tag="wt")
                nc.vector.tensor_scalar(out=wt[:], in0=ws[:], scalar1=0.2,
                                        scalar2=None, op0=ALU.mult)
                nc.vector.tensor_tensor(wt[:], wt[:], ws[:], op=ALU.max)
                nc.scalar.activation(wt[:], wt[:], AF.Exp)

                # weighted one-hot scatter: sw_h = sall * w_h (stride-0 bcast)
                swh = []
                for h in range(nhead):
                    wvh = wv[:, h:h + 1]
                    wview = AP(wvh.tensor, wvh.offset,
                               [wvh.ap[0], [nhead, CPB], [0, P]])
                    swt = pm.tile([P, CPB * P], bf16, tag=f"swh{h}")
                    nc.vector.tensor_tensor(swt[:], sall[:], wview,
                                            op=ALU.mult)
                    swh.append(swt)

                # single psum accumulation group across all j/h matmuls:
                # start only on the first, stop only on the very last.  Table
                # rows carry a literal 1.0 after each head's features, so one
                # matmul accumulates both the weighted feature sum and the
                # softmax denominator:
                # bp layout [f1 0:128 | sumw1 | f2 129:257 | sumw2] (2 heads)
                # or [f 0:256 | sumw] (1 head).
                HB = P + 1 if nhead == 2 else IN + 1
                bp = pp.tile([P, nhead * HB], f32, tag="bp")
                for j in range(NSW):
                    last = (j == NSW - 1)
                    selfc = (j == CPB)
                    for h in range(nhead):
                        if selfc:
                            sw = pm.tile([P, P], bf16, tag="sw")
                            nc.vector.tensor_scalar(
                                out=sw[:], in0=identb_sb[:],
                                scalar1=wt[:, h:h + 1],
                                scalar2=None, op0=ALU.mult)
                            lhsT = sw[:]
                        else:
                            lhsT = swh[h][:, j * P:(j + 1) * P]
                        c0, c1 = h * HB, (h + 1) * HB
                        rhs = win[:, c0:c1] if selfc else gt[:, j, c0:c1]
                        nc.tensor.matmul(bp[:, c0:c1], lhsT=lhsT,
                                         rhs=rhs,
                                         start=(j == 0 and h == 0),
                                         stop=(last and h == nhead - 1))

                # ---- block epilogue
                rec = po.tile([P, nhead], f32, tag="rec")
                for h in range(nhead):
                    nc.vector.reciprocal(rec[:, h:h + 1],
                                         bp[:, (h + 1) * HB - 1:
                                            (h + 1) * HB])
                ti = po.tile([P, IN], f32, tag="ti")
                if nhead == 2:
                    nc.scalar.activation(ti[:, 0:P], bp[:, 0:P], AF.Copy,
                                         scale=rec[:, 0:1])
                    nc.scalar.activation(ti[:, P:IN], bp[:, HB:HB + P],
                                         AF.Copy, scale=rec[:, 1:2])
                else:
                    nc.scalar.activation(ti[:], bp[:, 0:IN], AF.Copy,
                                         scale=rec[:, 0:1])
                nc.vector.tensor_tensor(ti[:], ti[:], b_sb[:], op=ALU.add)
                if lrelu_out:
                    # lrelu(x) = 0.01x + relu(0.99x)
                    tr = po.tile([P, IN], f32, tag="tr")
                    nc.scalar.activation(tr[:], ti[:], AF.Relu, scale=0.99)
                    nc.vector.scalar_tensor_tensor(
                        out=ti[:], in0=ti[:], scalar=0.01, in1=tr[:],
                        op0=ALU.mult, op1=ALU.add)
                for k in range(2):
                    tp = pt.tile([P, P], f32, tag="tp")
                    nc.tensor.transpose(tp[:], ti[:, k * P:(k + 1) * P],
                                        ident_sb[:])
                    nc.scalar.activation(out_t[:, k, blk * P:(blk + 1) * P],
                                         tp[:], AF.Copy)

                if layer == 1:
                    hp = ph.tile([P, AUG2], f32, tag="hp")
                    for k in range(2):
                        nc.tensor.matmul(
                            hp[:],
                            lhsT=out1T_sb[:, k, blk * P:(blk + 1) * P],
                            rhs=w2_sb[:, k, :], start=(k == 0), stop=(k == 1))
                    # L2 row: [h 0:256 | 1.0 | a_s f32 @129 | a_d f32 @130]
                    row2 = po.tile([P, ROWW], f16, tag="row2")
                    nc.scalar.activation(row2[:, 0:OUT], hp[:, 0:OUT], AF.Copy)
                    nc.vector.memset(row2[:, OUT:OUT + 2], 1.0)
                    r2f = row2[:].bitcast(f32)
                    nc.vector.tensor_copy(r2f[:, 129:131],
                                          hp[:, OUT:OUT + 2])
                    nc.sync.dma_start(
                        cc2_in[blk * P:(blk + 1) * P, 0:OUT + 6],
                        row2[:, 0:OUT + 6])

                if post_block is not None:
                    post_block(blk)

            for b in range(NBLK + 1):
                if b < NBLK:
                    stage_a(b)
                if b > 0:
                    stage_b(b - 1)

            estack.close()

        def _post1(blk):
            if blk == PB0 - 1:
                ag_piece(cc2_in, cc2_p0, 0)
            elif blk == NBLK - 1:
                ag_piece(cc2_in, cc2_p1, 1)

        edge_phase(1, post_block=_post1)

        # ====== P5 head, interleaved into edge phase 2 ======================
        hstack = ExitStack()
        hps = hstack.enter_context(
            tc.tile_pool(name="hps", bufs=1, space="PSUM"))
        hsb = hstack.enter_context(tc.tile_pool(name="hsb", bufs=2))
        sps = hstack.enter_context(
            tc.tile_pool(name="sps", bufs=1, space="PSUM"))
        hepi = hstack.enter_context(tc.tile_pool(name="hepi", bufs=2))

        NTL = []
        _st = 0
        while _st < CAP:
            _w = min(512, CAP - _st)
            NTL.append((_st, _w))
            _st += _w

        def head_slice(st, wdt):
            nump = sps.tile([KH, 512], f32, tag="nump")
            nrmp = sps.tile([KH, 512], f32, tag="nrmp")
            for k in range(KH):
                hp = hps.tile([P, 512], f32, tag="hp")
                for f in range(2):
                    nc.tensor.matmul(hp[:, 0:wdt],
                                     lhsT=g_sb[:, f, k * P:(k + 1) * P],
                                     rhs=h2fT_sb[:, f, st:st + wdt],
                                     start=(f == 0), stop=(f == 1))
                h16 = hsb.tile([P, 512], f16, tag="h16")
                sq16 = hsb.tile([P, 512], f16, tag="sq16")
                nc.vector.tensor_copy(h16[:, 0:wdt], hp[:, 0:wdt])
                nc.scalar.activation(sq16[:, 0:wdt], hp[:, 0:wdt], AF.Square)
                nc.tensor.matmul(nump[:, 0:wdt],
                                 lhsT=mu_sb[:, k * KH:(k + 1) * KH],
                                 rhs=h16[:, 0:wdt], start=(k == 0),
                                 stop=(k == KH - 1))
                nc.tensor.matmul(nrmp[:, 0:wdt],
                                 lhsT=on_sb[:, k * KH:(k + 1) * KH],
                                 rhs=sq16[:, 0:wdt], start=(k == 0),
                                 stop=(k == KH - 1))
            sq = hepi.tile([KH, 512], f32, tag="sqr")
            # sqrt(x) = exp(0.5*ln(x)) -- keeps ACT on the ln/exp table set
            nc.scalar.activation(sq[:, 0:wdt], nrmp[:, 0:wdt], AF.Ln)
            nc.scalar.activation(sq[:, 0:wdt], sq[:, 0:wdt], AF.Exp,
                                 scale=0.5)
            nc.vector.tensor_scalar(out=sq[:, 0:wdt], in0=sq[:, 0:wdt],
                                    scalar1=cmu_sb[:, 0:1], scalar2=1e-8,
                                    op0=ALU.mult, op1=ALU.max)
            nc.vector.reciprocal(sq[:, 0:wdt], sq[:, 0:wdt])
            res = hepi.tile([KH, 512], f32, tag="res")
            nc.vector.tensor_tensor(res[:, 0:wdt], nump[:, 0:wdt],
                                    sq[:, 0:wdt], op=ALU.mult)
            nc.sync.dma_start(outT[:, st:st + wdt], res[:, 0:wdt])

        _emitted = [0]

        def _post2(blk):
            done = (blk + 1) * P
            while _emitted[0] < len(NTL):
                st, wdt = NTL[_emitted[0]]
                if st + wdt > done:
                    break
                head_slice(st, wdt)
                _emitted[0] += 1

        edge_phase(2, post_block=_post2)
        while _emitted[0] < len(NTL):
            st, wdt = NTL[_emitted[0]]
            head_slice(st, wdt)
            _emitted[0] += 1
        hstack.close()

    nc.compile()
    return nc


# ======================= host-side preparation ==============================

def _wrap16(flat):
    """idx flat [n] -> wrapped int16 [128, n//16]; pos i -> (i%16, i//16),
    replicated across the 8 Q7-core stripes."""
    n = len(flat)
    out = np.zeros((P, n // 16), np.int16)
    cols = np.arange(n) // 16
    rows = np.arange(n) % 16
    for r in range(8):
        out[r * 16 + rows, cols] = flat
    return out


def _balance_bins(deg, nbins, cap):
    """Greedy multiway partition: assign nodes to bins balancing total degree,
    each bin holding at most `cap` nodes.  Returns bin id per node."""
    import heapq
    n = len(deg)
    order = np.argsort(-deg, kind="stable")
    binid = np.empty(n, np.int32)
    counts = np.zeros(nbins, np.int32)
    heap = [(0, b) for b in range(nbins)]
    heapq.heapify(heap)
    for nd in order:
        while True:
            load, b = heapq.heappop(heap)
            if counts[b] < cap:
                break
        binid[nd] = b
        counts[b] += 1
        if counts[b] < cap:
            heapq.heappush(heap, (load + int(deg[nd]), b))
    return binid


def prep_host(x, edge_index, W1, a_src1, a_dst1, b1, W2, a_src2, a_dst2, b2,
              g, mu, world=8):
    import ml_dtypes
    x16 = np.asarray(x, np.float32).astype(np.float16)
    N = x16.shape[0]
    NBLK = int(np.ceil(N / world / P))
    CAP = NBLK * P
    nbins = world * NBLK
    PB0 = (NBLK + 1) // 2
    PB1 = NBLK - PB0

    src = np.asarray(edge_index[0]).astype(np.int64)
    dst = np.asarray(edge_index[1]).astype(np.int64)

    # --- balanced global node -> (core, block, slot) assignment
    deg = np.bincount(dst, minlength=N)
    binid = _balance_bins(deg, nbins, P)
    order = np.lexsort((np.arange(N), binid))
    # local slot position within the core's shard
    lpos = np.empty(N, np.int64)
    nxt = np.arange(nbins, dtype=np.int64) * P
    for nd in order:
        b = binid[nd]
        lpos[nd] = nxt[b]
        nxt[b] += 1
    node_core = binid // NBLK
    node_blk = binid % NBLK
    lpos -= node_core * CAP              # position within own core [0, CAP)

    # global cc table position: AllGather piece-major
    # piece0 rows: [core, blocks 0:PB0]; piece1: [core, blocks PB0:NBLK]
    in_p1 = node_blk >= PB0
    gpos = np.where(
        ~in_p1,
        node_core * (PB0 * P) + lpos,
        world * PB0 * P + node_core * (PB1 * P) + (lpos - PB0 * P))

    # per-core list of node ids in shard slot order (-1 = empty slot)
    idxmaps = []
    for c in range(world):
        m = np.full(CAP, -1, np.int64)
        mask = node_core == c
        m[lpos[mask]] = np.nonzero(mask)[0]
        idxmaps.append(m)

    # --- edges grouped by (core, block) of dst
    ecore = node_core[dst]
    eblk = node_blk[dst]
    gkey = ecore * NBLK + eblk
    gorder = np.argsort(gkey, kind="stable")
    srcg, dstg, gkeyg = src[gorder], dst[gorder], gkey[gorder]
    starts = np.concatenate(
        [[0], np.cumsum(np.bincount(gkeyg, minlength=nbins))])

    ed = {}
    CPL = CPH = 1
    for c in range(world):
        for b in range(NBLK):
            gid = c * NBLK + b
            es = srcg[starts[gid]:starts[gid + 1]]
            eds = dstg[starts[gid]:starts[gid + 1]]
            dloc = (lpos[eds] - b * P).astype(np.int64)
            tl = gpos[es]
            lo = tl < world * PB0 * P
            ed[(c, b)] = (es, tl, lo, dloc)
            CPL = max(CPL, int(np.ceil(lo.sum() / P)))
            CPH = max(CPH, int(np.ceil((~lo).sum() / P)))

    cfg = CFG(N=N, W=world, NBLK=NBLK, CPL=CPL, CPH=CPH, idxmaps=idxmaps)
    CPB = cfg.CPB
    DCL, DCH, DCT = cfg.DCL, cfg.DCH, cfg.DCT
    ar128 = np.arange(P, dtype=np.int64)

    def build_core(c):
        isd = np.zeros((P, NBLK * CPB * 8), np.int16)
        sth = np.zeros((P, NBLK * CPB * P), ml_dtypes.bfloat16)
        salh = np.zeros((P, NBLK * CPB * P), ml_dtypes.bfloat16)
        srcs = np.zeros((NBLK, DCT * P), np.int64)    # dense-chunk x rows
        for b in range(NBLK):
            es, tl, lo, dloc = ed[(c, b)]
            fl = np.zeros(CPB * P, np.int64)      # slot -> table idx (pad 0)
            fd = np.full(CPB * P, -1, np.int64)   # slot -> dst_local (pad -1)
            fs = np.zeros(CPB * P, np.int64)      # slot -> src node id
            ilo = np.where(lo)[0]
            ihi = np.where(~lo)[0]
            fl[:len(ilo)] = tl[ilo]
            fd[:len(ilo)] = dloc[ilo]
            fs[:len(ilo)] = es[ilo]
            fl[CPL * P:CPL * P + len(ihi)] = tl[ihi] - world * PB0 * P
            fd[CPL * P:CPL * P + len(ihi)] = dloc[ihi]
            fs[CPL * P:CPL * P + len(ihi)] = es[ihi]
            cb8 = b * CPB * 8
            isd[:, cb8:cb8 + CPB * 8] = _wrap16(fl)
            # one-hots from fd [CPB, P]
            fdm = fd.reshape(CPB, P)
            oh = (fdm[:, :, None] == ar128)                 # [j, e, d]
            cbp = b * CPB * P
            sth[:, cbp:cbp + CPB * P] = \
                oh.transpose(2, 0, 1).reshape(P, CPB * P)   # st[d,(j,e)]
            salh[:, cbp:cbp + CPB * P] = \
                oh.transpose(1, 0, 2).reshape(P, CPB * P)   # sall[e,(j,d)]
            # dense chunk sources: lo [CPL-DCL:CPL], hi [CPB-DCH:CPB]
            fsm = fs.reshape(CPB, P)
            if DCL:
                srcs[b, 0:DCL * P] = fsm[CPL - DCL:CPL].ravel()
            if DCH:
                srcs[b, DCL * P:DCT * P] = fsm[CPB - DCH:CPB].ravel()
        xs = x16[srcs.ravel()]                    # [NBLK*DCT*P, IN]
        xsT = np.ascontiguousarray(
            xs.reshape(NBLK, DCT, P, 2, P).transpose(4, 0, 1, 3, 2))
        return isd, sth, salh, xsT

    # weights
    W1 = np.asarray(W1, np.float32)
    W2 = np.asarray(W2, np.float32)
    W1r = W1.reshape(H1, MD, IN)
    Ps1 = np.einsum("hdi,hd->ih", W1r, np.asarray(a_src1, np.float32))
    Pd1 = np.einsum("hdi,hd->ih", W1r, np.asarray(a_dst1, np.float32))
    W1aug = np.concatenate([W1.T, Ps1, Pd1], axis=1)
    Ps2 = W2.T @ np.asarray(a_src2, np.float32)[0][:, None]
    Pd2 = W2.T @ np.asarray(a_dst2, np.float32)[0][:, None]
    W2aug = np.concatenate([W2.T, Ps2, Pd2], axis=1)
    AUG1, AUG2 = IN + 4, IN + 2
    w1s = W1aug.reshape(2, P, AUG1).transpose(1, 0, 2).astype(np.float16)
    w2s = W2aug.reshape(2, P, AUG2).transpose(1, 0, 2).astype(np.float16)

    gm = np.asarray(g, np.float32)
    gsd = gm.reshape(2, P, KH * P).transpose(1, 0, 2).astype(np.float16)
    mu = np.asarray(mu, np.float32)
    mus = np.zeros((P, KH * KH), np.float16)
    onesd = np.zeros((P, KH * KH), np.float16)
    for k in range(KH):
        mus[:, k * KH + k] = mu[k, :]
        onesd[:, k * KH + k] = 1.0
    cmu = np.linalg.norm(mu, axis=1)[:, None].astype(np.float32)
    b1b = np.broadcast_to(np.asarray(b1, np.float32), (P, HID)).copy()
    b2b = np.broadcast_to(np.asarray(b2, np.float32), (P, OUT)).copy()
    ident = np.eye(P, dtype=np.float32)
    identb = np.eye(P, dtype=ml_dtypes.bfloat16)

    shared = dict(w1s=w1s, w2s=w2s, gs=gsd, mus=mus, onesd=onesd, cmu=cmu,
                  b1b=b1b, b2b=b2b, ident=ident, identb=identb)
    in_maps = []
    for c in range(world):
        m = idxmaps[c]
        own = np.where(m >= 0, m, 0)
        xo = x16[own]
        xo[m < 0] = 0
        xoT = np.ascontiguousarray(
            xo.reshape(NBLK, P, 2, P).transpose(3, 0, 2, 1))
        isd_c, st_c, sal_c, xsT_c = build_core(c)
        mm = dict(shared)
        mm.update(xoTi=xoT, xsTi=xsT_c, isd=isd_c, std=st_c, sald=sal_c)
        in_maps.append(mm)
    return cfg, in_maps


def assemble(cfg, outs):
    N = cfg.N
    full = np.zeros((N, KH), np.float32)
    for c in range(cfg.W):
        o = np.asarray(outs[c]["outT"])      # [KH, SHARD_CAP]
        m = cfg.idxmaps[c]
        valid = m >= 0
        full[m[valid], :] = o[:, valid].T
    return full


_CACHE = {}


def kernel(**inputs):
    world = 8
    cfg, in_maps = prep_host(world=world, **inputs)
    key = (cfg.N, cfg.W, cfg.CPL, cfg.CPH)
    if key not in _CACHE:
        _CACHE[key] = build_program(cfg)
    nc = _CACHE[key]

    from concourse.bass_utils import run_bass_kernel_spmd
    res = run_bass_kernel_spmd(nc, in_maps, core_ids=list(range(world)))
    return assemble(cfg, res.results)


# revision 6
# speedup vs baseline: 1.3618x; 1.0636x over previous
"""Trainium2 Bass kernel for nn_NodeInference (2-layer GAT + cosine head).

v4 design (SPMD over 8 cores, dst-node sharding, hybrid gather/dense):
  Host globally re-assigns nodes to (core, block) bins, balancing per-block
  in-degree.  Both GAT layers share ONE edge-slot layout (chunks of 128
  edges per dst block, split lo/hi by global table row for int16 gather
  indices), so the per-edge one-hot matrices are built once on the host and
  shipped:
     std  [e-transposed]  st[d,(j,e)]  = (dloc[j,e]==d)   (ad lookup lhsT)
     sald                 sall[e,(j,d)] = (dloc[j,e]==d)  (scatter base)
  This removes the per-block PE broadcast + DVE is_equal chains of v2.

  P1  sharded dense: each core computes h1aug only for its OWN 6272 nodes
      -> cc1_in rows [h1|1|h2|1|as f32 x2|ad f32 x2] (768B)
  AG1 AllGather cc1_in -> cc1_out (global h1 table), in 2 pieces
  P2  edge phase 1 per dst block: chunks are HYBRID:
      - gather chunks: dma_gather rows from cc1_out (GpSimd)
      - dense chunks:  gt[:,j] = x[src_e] @ W1aug on the PE (x[src_e] is a
        host input, shipped pre-arranged per edge slot in xsTi) -- trades
        GpSimd descriptor-generation time for PE time to balance engines
      - a_d per edge = st_j^T @ adwin;  w_e = exp(min(lrelu(a_s+a_d,.2),30))
      - scatter: bp += (sall*w_h)_j^T @ rows_j  (rows carry literal 1.0 so
        the same matmul accumulates the softmax denominator)
      - epilogue -> out1T; h2aug rows -> cc2_in
      Blocks are software-pipelined (stage A: dma/dense/gather/ad of block
      b+1 emitted before stage B: ew/swh/scatter/epilogue of block b) to
      avoid in-order PE stalls.
  AG2 AllGather cc2_in -> cc2_out in 2 pieces, piece 0 issued mid-phase
  P4  edge phase 2: all chunks gathered from cc2_out (content is
      device-computed, so the host x-trick cannot apply)
  P5  head: cos sim vs mu -> outT [8, SHARD_CAP], interleaved into P4
Host scatters per-core outT into the full output via the assignment map.
"""

import sys
from dataclasses import dataclass, field
from contextlib import ExitStack

if "/opt/trn_rl_repo" not in sys.path:
    sys.path.insert(0, "/opt/trn_rl_repo")

import numpy as np

import concourse.bacc as bacc
import concourse.bass as bass
import concourse.mybir as mybir
import concourse.tile as tile
from concourse.bass import AP

P = 128
IN = 256          # input feature dim
H1 = 2            # layer-1 heads
HID = 256         # layer-1 output dim (2*128, concat)
OUT = 256         # layer-2 output dim
KH, MD = 8, 128   # cosine head shape
ROWW = 384        # fp16 cols per packed table row (768B)
HALF = 32768      # int16 table-half split
DENSE_LO = 4      # layer-1 lo chunks computed on the PE instead of gathered
DENSE_HI = 3      # layer-1 hi chunks computed on the PE
AF = mybir.ActivationFunctionType
ALU = mybir.AluOpType
DT = mybir.dt


@dataclass
class CFG:
    N: int
    W: int              # world size
    NBLK: int           # dst blocks (128 dsts) per core
    CPL: int            # lo-half chunks per block
    CPH: int
    idxmaps: object = field(default=None, repr=False)

    @property
    def SHARD_CAP(self):
        return self.NBLK * P

    @property
    def CPB(self):
        return self.CPL + self.CPH

    @property
    def DCL(self):
        return min(DENSE_LO, self.CPL)

    @property
    def DCH(self):
        return min(DENSE_HI, self.CPH)

    @property
    def DCT(self):
        return self.DCL + self.DCH

    @property
    def PB0(self):       # blocks in AllGather piece 0 (int16 row limit)
        return min(self.NBLK, HALF // (self.W * P))


def build_program(cfg: CFG):
    nc = bacc.Bacc("TRN2", target_bir_lowering=False, debug=False)
    W, NBLK = cfg.W, cfg.NBLK
    AUG1, AUG2 = IN + 4, IN + 2
    f16, bf16, f32 = DT.float16, DT.bfloat16, DT.float32
    i16 = DT.int16
    CPB, CPL, CPH = cfg.CPB, cfg.CPL, cfg.CPH
    DCL, DCH = cfg.DCL, cfg.DCH
    PB0, PB1 = cfg.PB0, cfg.NBLK - cfg.PB0
    CAP = cfg.SHARD_CAP

    with tile.TileContext(nc) as tc, ExitStack() as stack:
        dram = stack.enter_context(
            tc.tile_pool(name="dram", bufs=1, space="DRAM"))

        def din(name, shape, dtype):
            return dram.tile(shape, dtype, kind="ExternalInput", name=name,
                             uniquify=False)

        xoTi = din("xoTi", [P, NBLK, 2, P], f16)
        xsTi = din("xsTi", [P, NBLK, cfg.DCT, 2, P], f16)
        isd = din("isd", [P, NBLK * CPB * 8], i16)
        std = din("std", [P, NBLK * CPB * P], bf16)
        sald = din("sald", [P, NBLK * CPB * P], bf16)
        w1s = din("w1s", [P, 2, AUG1], f16)
        w2s = din("w2s", [P, 2, AUG2], f16)
        gsd = din("gs", [P, 2, KH * P], f16)
        mus = din("mus", [P, KH * KH], f16)       # block-diag mu^T
        ond = din("onesd", [P, KH * KH], f16)     # block-diag ones
        cmu = din("cmu", [KH, 1], f32)
        b1d = din("b1b", [P, HID], f32)
        b2d = din("b2b", [P, OUT], f32)
        idn = din("ident", [P, P], f32)
        idb = din("identb", [P, P], bf16)
        outT = dram.tile([KH, CAP], f32, kind="ExternalOutput",
                         name="outT", uniquify=False)

        shsp = "Shared" if W > 1 else "Local"
        cc1_in = dram.tile([CAP, ROWW], f16, name="cc1_in")
        cc1_p0 = dram.tile([W * PB0 * P, ROWW], f16, name="cc1_p0",
                           addr_space=shsp)
        cc1_p1 = dram.tile([W * PB1 * P, ROWW], f16, name="cc1_p1",
                           addr_space=shsp)
        cc2_in = dram.tile([CAP, ROWW], f16, name="cc2_in")
        cc2_p0 = dram.tile([W * PB0 * P, ROWW], f16, name="cc2_p0",
                           addr_space=shsp)
        cc2_p1 = dram.tile([W * PB1 * P, ROWW], f16, name="cc2_p1",
                           addr_space=shsp)

        consts = stack.enter_context(tc.tile_pool(name="consts", bufs=1))
        w1_sb = consts.tile([P, 2, AUG1], f16)
        w2_sb = consts.tile([P, 2, AUG2], f16)
        g_sb = consts.tile([P, 2, KH * P], f16)
        mu_sb = consts.tile([P, KH * KH], f16)
        on_sb = consts.tile([P, KH * KH], f16)
        cmu_sb = consts.tile([KH, 1], f32)
        b1_sb = consts.tile([P, HID], f32)
        b2_sb = consts.tile([P, OUT], f32)
        ident_sb = consts.tile([P, P], f32)
        identb_sb = consts.tile([P, P], bf16)
        out1T_sb = consts.tile([P, 2, CAP], f16)
        h2fT_sb = consts.tile([P, 2, CAP], f16)

        for dst, src in [(w1_sb, w1s), (w2_sb, w2s), (g_sb, gsd),
                         (mu_sb, mus), (on_sb, ond), (cmu_sb, cmu),
                         (b1_sb, b1d), (b2_sb, b2d),
                         (ident_sb, idn), (identb_sb, idb)]:
            nc.sync.dma_start(dst[:], src[:])

        def ag_piece(cin, cout_p, pc):
            """AllGather piece pc (0/1) of cin into its own Shared tensor."""
            lr = (0, PB0 * P) if pc == 0 else (PB0 * P, CAP)
            nc.gpsimd.collective_compute(
                "AllGather", ALU.bypass,
                replica_groups=[list(range(W))],
                ins=[cin[lr[0]:lr[1], :]], outs=[cout_p[:, :]])

        # ================= P1: sharded dense (own nodes only) ===============
        with tc.tile_pool(name="p1x", bufs=3) as p1x, \
             tc.tile_pool(name="p1ps", bufs=3, space="PSUM") as p1ps, \
             tc.tile_pool(name="p1row", bufs=3) as p1row:
            for blk in range(NBLK):
                xo = p1x.tile([P, 2, P], f16, tag="xo")
                nc.sync.dma_start(xo[:], xoTi[:, blk, :, :])
                ps = p1ps.tile([P, AUG1], f32, tag="ps")
                for k in range(2):
                    nc.tensor.matmul(ps[:], lhsT=xo[:, k, :],
                                     rhs=w1_sb[:, k, :],
                                     start=(k == 0), stop=(k == 1))
                # row: [h1 0:128 | 1.0 | h2 129:257 | 1.0 | as f32 x2 | ad..]
                row = p1row.tile([P, 272], f16, tag="row")
                rf32 = row[:].bitcast(f32)
                nc.scalar.activation(row[:, 0:P], ps[:, 0:P], AF.Copy)
                nc.scalar.activation(row[:, P + 1:IN + 1],
                                     ps[:, P:IN], AF.Copy)
                nc.vector.tensor_copy(rf32[:, 129:133], ps[:, IN:IN + 4])
                nc.vector.memset(row[:, P:P + 1], 1.0)
                nc.vector.memset(row[:, IN + 1:IN + 2], 1.0)
                nc.sync.dma_start(
                    cc1_in[blk * P:(blk + 1) * P, 0:IN + 10],
                    row[:, 0:IN + 10])
                if blk == PB0 - 1:
                    ag_piece(cc1_in, cc1_p0, 0)
            ag_piece(cc1_in, cc1_p1, 1)

        # ================= edge phases ======================================
        def edge_phase(layer, post_block=None):
            if layer == 1:
                nhead = 2
                b_sb, out_t, lrelu_out = b1_sb, out1T_sb, True
                as_off, ad_off = 129, 131   # f32 col offsets in table rows
                cin, tab_lo, tab_hi = cc1_in, cc1_p0[:, :], cc1_p1[:, :]
                # chunk roles: lo [0:CPL-DCL] gather, [CPL-DCL:CPL] dense
                #              hi [CPL:CPB-DCH] gather, [CPB-DCH:CPB] dense
                g_rngs = [(0, CPL - DCL), (CPL, CPB - DCH)]
                d_rngs = [(CPL - DCL, CPL, 0), (CPB - DCH, CPB, DCL)]
            else:
                nhead = 1
                b_sb, out_t, lrelu_out = b2_sb, h2fT_sb, False
                as_off, ad_off = 129, 130
                cin, tab_lo, tab_hi = cc2_in, cc2_p0[:, :], cc2_p1[:, :]
                g_rngs = [(0, CPL), (CPL, CPB)]
                d_rngs = []

            estack = ExitStack()
            pi = estack.enter_context(
                tc.tile_pool(name=f"idx{layer}", bufs=3))
            pso = estack.enter_context(
                tc.tile_pool(name=f"soh{layer}", bufs=3))
            pg = estack.enter_context(
                tc.tile_pool(name=f"gath{layer}", bufs=3))
            pw = estack.enter_context(
                tc.tile_pool(name=f"win{layer}", bufs=3))
            px = (estack.enter_context(tc.tile_pool(name="dsx", bufs=3))
                  if d_rngs else None)
            pps = (estack.enter_context(
                tc.tile_pool(name="dsps", bufs=2, space="PSUM"))
                if d_rngs else None)
            pad_ = estack.enter_context(
                tc.tile_pool(name=f"adps{layer}", bufs=2, space="PSUM"))
            pe_ = estack.enter_context(tc.tile_pool(name=f"ew{layer}", bufs=3))
            pm = estack.enter_context(tc.tile_pool(name=f"sw{layer}", bufs=2))
            pp = estack.enter_context(
                tc.tile_pool(name=f"bps{layer}", bufs=2, space="PSUM"))
            pt = estack.enter_context(
                tc.tile_pool(name=f"tps{layer}", bufs=1, space="PSUM"))
            po = estack.enter_context(tc.tile_pool(name=f"epi{layer}", bufs=2))
            ph = estack.enter_context(
                tc.tile_pool(name=f"h2ps{layer}", bufs=1, space="PSUM"))

            NSW = CPB + 1   # chunks + self-loop virtual chunk
            state = {}

            def stage_a(blk):
                cb8 = blk * CPB * 8
                isdt = pi.tile([P, CPB * 8], i16, tag="isd")
                nc.sync.dma_start(isdt[:], isd[:, cb8:cb8 + CPB * 8])
                st = pso.tile([P, CPB * P], bf16, tag="st")
                nc.sync.dma_start(st[:], std[:, blk * CPB * P:
                                             (blk + 1) * CPB * P])
                sall = pso.tile([P, CPB * P], bf16, tag="sall")
                nc.sync.dma_start(sall[:], sald[:, blk * CPB * P:
                                                (blk + 1) * CPB * P])
                win = pw.tile([P, ROWW], f16, tag="win")
                nc.sync.dma_start(win[:], cin[blk * P:(blk + 1) * P, :])
                winf = win[:].bitcast(f32)

                gt = pg.tile([P, CPB, ROWW], f16, tag="gt")
                gtf = gt[:].bitcast(f32)
                # gather chunks
                MXC = 8
                for r0, r1 in g_rngs:
                    for c0 in range(r0, r1, MXC):
                        c1 = min(c0 + MXC, r1)
                        nc.gpsimd.dma_gather(
                            gt[:, c0:c1, :],
                            tab_lo if r0 == 0 else tab_hi,
                            isdt[:, c0 * 8:c1 * 8],
                            (c1 - c0) * P, (c1 - c0) * P, ROWW)
                # dense chunks (layer 1 only)
                if d_rngs:
                    xt = px.tile([P, cfg.DCT, 2, P], f16, tag="xt")
                    nc.sync.dma_start(xt[:], xsTi[:, blk, :, :, :])
                    for r0, r1, xb in d_rngs:
                        for j in range(r0, r1):
                            ps = pps.tile([P, AUG1], f32, tag="ps")
                            xi = xb + (j - r0)
                            for k in range(2):
                                nc.tensor.matmul(ps[:], lhsT=xt[:, xi, k, :],
                                                 rhs=w1_sb[:, k, :],
                                                 start=(k == 0), stop=(k == 1))
                            nc.scalar.activation(gt[:, j, 0:P], ps[:, 0:P],
                                                 AF.Copy)
                            nc.scalar.activation(gt[:, j, P + 1:IN + 1],
                                                 ps[:, P:IN], AF.Copy)
                            nc.vector.tensor_copy(gtf[:, j, 129:131],
                                                  ps[:, IN:IN + 2])
                        nc.vector.memset(gt[:, r0:r1, P:P + 1], 1.0)
                        nc.vector.memset(gt[:, r0:r1, IN + 1:IN + 2], 1.0)

                # a_d per edge: ad_ps[e, (j,h)] = st_j^T @ adwin
                adw16 = pe_.tile([P, nhead], f16, tag="adw16")
                nc.vector.tensor_copy(adw16[:],
                                      winf[:, ad_off:ad_off + nhead])
                ad_ps = pad_.tile([P, CPB * nhead], f32, tag="adps")
                for j in range(CPB):
                    nc.tensor.matmul(ad_ps[:, j * nhead:(j + 1) * nhead],
                                     lhsT=st[:, j * P:(j + 1) * P],
                                     rhs=adw16[:], start=(j == 0),
                                     stop=(j == CPB - 1))
                state[blk] = (gt, gtf, win, winf, sall, ad_ps)

            def stage_b(blk):
                gt, gtf, win, winf, sall, ad_ps = state.pop(blk)
                # edge weights w = exp(lrelu(a_s + a_d, .2))
                ew = pe_.tile([P, CPB * nhead], f32, tag="ew")
                wv = pe_.tile([P, CPB * nhead], bf16, tag="wv")
                as_v = gtf[:, :, as_off:as_off + nhead]
                nc.vector.tensor_tensor(ew[:], as_v, ad_ps[:], op=ALU.add)
                ewl = pe_.tile([P, CPB * nhead], f32, tag="ewl")
                nc.vector.tensor_scalar(out=ewl[:], in0=ew[:], scalar1=0.2,
                                        scalar2=None, op0=ALU.mult)
                nc.vector.tensor_tensor(ewl[:], ewl[:], ew[:], op=ALU.max)
                nc.scalar.activation(wv[:], ewl[:], AF.Exp)

                # self-loop weights from the window rows
                ws = pe_.tile([P, nhead], f32, tag="ws")
                nc.vector.tensor_tensor(
                    ws[:], winf[:, as_off:as_off + nhead],
                    winf[:, ad_off:ad_off + nhead], op=ALU.add)
                wt = pe_.tile([P, nhead], f32, tag="wt")
                nc.vector.tensor_scalar(out=wt[:], in0=ws[:], scalar1=0.2,
                                        scalar2=None, op0=ALU.mult)
                nc.vector.tensor_tensor(wt[:], wt[:], ws[:], op=ALU.max)
                nc.scalar.activation(wt[:], wt[:], AF.Exp)

                # weighted one-hot scatter: sw_h = sall * w_h (stride-0 bcast)
                swh = []
                for h in range(nhead):
                    wvh = wv[:, h:h + 1]
                    wview = AP(wvh.tensor, wvh.offset,
                               [wvh.ap[0], [nhead, CPB], [0, P]])
                    swt = pm.tile([P, CPB * P], bf16, tag=f"swh{h}")
                    nc.vector.tensor_tensor(swt[:], sall[:], wview,
                                            op=ALU.mult)
                    swh.append(swt)

                # single psum accumulation group across all j/h matmuls:
                # start only on the first, stop only on the very last.  Table
                # rows carry a literal 1.0 after each head's features, so one
                # matmul accumulates both the weighted feature sum and the
                # softmax denominator:
                # bp layout [f1 0:128 | sumw1 | f2 129:257 | sumw2] (2 heads)
                # or [f 0:256 | sumw] (1 head).
                HB = P + 1 if nhead == 2 else IN + 1
                bp = pp.tile([P, nhead * HB], f32, tag="bp")
                for j in range(NSW):
                    last = (j == NSW - 1)
                    selfc = (j == CPB)
                    for h in range(nhead):
                        if selfc:
                            sw = pm.tile([P, P], bf16, tag="sw")
                            nc.vector.tensor_scalar(
                                out=sw[:], in0=identb_sb[:],
                                scalar1=wt[:, h:h + 1],
                                scalar2=None, op0=ALU.mult)
                            lhsT = sw[:]
                        else:
                            lhsT = swh[h][:, j * P:(j + 1) * P]
                        c0, c1 = h * HB, (h + 1) * HB
                        rhs = win[:, c0:c1] if selfc else gt[:, j, c0:c1]
                        nc.tensor.matmul(bp[:, c0:c1], lhsT=lhsT,
                                         rhs=rhs,
                                         start=(j == 0 and h == 0),
                                         stop=(last and h == nhead - 1))

                # ---- block epilogue
                rec = po.tile([P, nhead], f32, tag="rec")
                for h in range(nhead):
                    nc.vector.reciprocal(rec[:, h:h + 1],
                                         bp[:, (h + 1) * HB - 1:
                                            (h + 1) * HB])
                ti = po.tile([P, IN], f32, tag="ti")
                if nhead == 2:
                    nc.scalar.activation(ti[:, 0:P], bp[:, 0:P], AF.Copy,
                                         scale=rec[:, 0:1])
                    nc.scalar.activation(ti[:, P:IN], bp[:, HB:HB + P],
                                         AF.Copy, scale=rec[:, 1:2])
                else:
                    nc.scalar.activation(ti[:], bp[:, 0:IN], AF.Copy,
                                         scale=rec[:, 0:1])
                nc.vector.tensor_tensor(ti[:], ti[:], b_sb[:], op=ALU.add)
                if lrelu_out:
                    # lrelu(x) = 0.01x + relu(0.99x)
                    tr = po.tile([P, IN], f32, tag="tr")
                    nc.scalar.activation(tr[:], ti[:], AF.Relu, scale=0.99)
                    nc.vector.scalar_tensor_tensor(
                        out=ti[:], in0=ti[:], scalar=0.01, in1=tr[:],
                        op0=ALU.mult, op1=ALU.add)
                for k in range(2):
                    tp = pt.tile([P, P], f32, tag="tp")
                    nc.tensor.transpose(tp[:], ti[:, k * P:(k + 1) * P],
                                        ident_sb[:])
                    nc.scalar.activation(out_t[:, k, blk * P:(blk + 1) * P],
                                         tp[:], AF.Copy)

                if layer == 1:
                    hp = ph.tile([P, AUG2], f32, tag="hp")
                    for k in range(2):
                        nc.tensor.matmul(
                            hp[:],
                            lhsT=out1T_sb[:, k, blk * P:(blk + 1) * P],
                            rhs=w2_sb[:, k, :], start=(k == 0), stop=(k == 1))
                    # L2 row: [h 0:256 | 1.0 | a_s f32 @129 | a_d f32 @130]
                    row2 = po.tile([P, ROWW], f16, tag="row2")
                    nc.scalar.activation(row2[:, 0:OUT], hp[:, 0:OUT], AF.Copy)
                    nc.vector.memset(row2[:, OUT:OUT + 2], 1.0)
                    r2f = row2[:].bitcast(f32)
                    nc.vector.tensor_copy(r2f[:, 129:131],
                                          hp[:, OUT:OUT + 2])
                    nc.sync.dma_start(
                        cc2_in[blk * P:(blk + 1) * P, 0:OUT + 6],
                        row2[:, 0:OUT + 6])

                if post_block is not None:
                    post_block(blk)

            for b in range(NBLK + 1):
                if b < NBLK:
                    stage_a(b)
                if b > 0:
                    stage_b(b - 1)

            estack.close()

        def _post1(blk):
            if blk == PB0 - 1:
                ag_piece(cc2_in, cc2_p0, 0)
            elif blk == NBLK - 1:
                ag_piece(cc2_in, cc2_p1, 1)

        edge_phase(1, post_block=_post1)

        # ====== P5 head, interleaved into edge phase 2 ======================
        hstack = ExitStack()
        hps = hstack.enter_context(
            tc.tile_pool(name="hps", bufs=1, space="PSUM"))
        hsb = hstack.enter_context(tc.tile_pool(name="hsb", bufs=2))
        sps = hstack.enter_context(
            tc.tile_pool(name="sps", bufs=1, space="PSUM"))
        hepi = hstack.enter_context(tc.tile_pool(name="hepi", bufs=2))

        NTL = []
        _st = 0
        while _st < CAP:
            _w = min(512, CAP - _st)
            NTL.append((_st, _w))
            _st += _w

        def head_slice(st, wdt):
            nump = sps.tile([KH, 512], f32, tag="nump")
            nrmp = sps.tile([KH, 512], f32, tag="nrmp")
            for k in range(KH):
                hp = hps.tile([P, 512], f32, tag="hp")
                for f in range(2):
                    nc.tensor.matmul(hp[:, 0:wdt],
                                     lhsT=g_sb[:, f, k * P:(k + 1) * P],
                                     rhs=h2fT_sb[:, f, st:st + wdt],
                                     start=(f == 0), stop=(f == 1))
                h16 = hsb.tile([P, 512], f16, tag="h16")
                sq16 = hsb.tile([P, 512], f16, tag="sq16")
                nc.vector.tensor_copy(h16[:, 0:wdt], hp[:, 0:wdt])
                nc.scalar.activation(sq16[:, 0:wdt], hp[:, 0:wdt], AF.Square)
                nc.tensor.matmul(nump[:, 0:wdt],
                                 lhsT=mu_sb[:, k * KH:(k + 1) * KH],
                                 rhs=h16[:, 0:wdt], start=(k == 0),
                                 stop=(k == KH - 1))
                nc.tensor.matmul(nrmp[:, 0:wdt],
                                 lhsT=on_sb[:, k * KH:(k + 1) * KH],
                                 rhs=sq16[:, 0:wdt], start=(k == 0),
                                 stop=(k == KH - 1))
            sq = hepi.tile([KH, 512], f32, tag="sqr")
            # sqrt(x) = exp(0.5*ln(x)) -- keeps ACT on the ln/exp table set
            nc.scalar.activation(sq[:, 0:wdt], nrmp[:, 0:wdt], AF.Ln)
            nc.scalar.activation(sq[:, 0:wdt], sq[:, 0:wdt], AF.Exp,
                                 scale=0.5)
            nc.vector.tensor_scalar(out=sq[:, 0:wdt], in0=sq[:, 0:wdt],
                                    scalar1=cmu_sb[:, 0:1], scalar2=1e-8,
                                    op0=ALU.mult, op1=ALU.max)
            nc.vector.reciprocal(sq[:, 0:wdt], sq[:, 0:wdt])
            res = hepi.tile([KH, 512], f32, tag="res")
            nc.vector.tensor_tensor(res[:, 0:wdt], nump[:, 0:wdt],
                                    sq[:, 0:wdt], op=ALU.mult)
            nc.sync.dma_start(outT[:, st:st + wdt], res[:, 0:wdt])

        _emitted = [0]

        def _post2(blk):
            done = (blk + 1) * P
            while _emitted[0] < len(NTL):
                st, wdt = NTL[_emitted[0]]
                if st + wdt > done:
                    break
                head_slice(st, wdt)
                _emitted[0] += 1

        edge_phase(2, post_block=_post2)
        while _emitted[0] < len(NTL):
            st, wdt = NTL[_emitted[0]]
            head_slice(st, wdt)
            _emitted[0] += 1
        hstack.close()

    nc.compile()
    return nc


# ======================= host-side preparation ==============================

def _wrap16(flat):
    """idx flat [n] -> wrapped int16 [128, n//16]; pos i -> (i%16, i//16),
    replicated across the 8 Q7-core stripes."""
    n = len(flat)
    out = np.zeros((P, n // 16), np.int16)
    cols = np.arange(n) // 16
    rows = np.arange(n) % 16
    for r in range(8):
        out[r * 16 + rows, cols] = flat
    return out


def _balance_bins(deg, nbins, cap):
    """Greedy multiway partition: assign nodes to bins balancing total degree,
    each bin holding at most `cap` nodes.  Returns bin id per node."""
    import heapq
    n = len(deg)
    order = np.argsort(-deg, kind="stable")
    binid = np.empty(n, np.int32)
    counts = np.zeros(nbins, np.int32)
    heap = [(0, b) for b in range(nbins)]
    heapq.heapify(heap)
    for nd in order:
        while True:
            load, b = heapq.heappop(heap)
            if counts[b] < cap:
                break
        binid[nd] = b
        counts[b] += 1
        if counts[b] < cap:
            heapq.heappush(heap, (load + int(deg[nd]), b))
    return binid


def prep_host(x, edge_index, W1, a_src1, a_dst1, b1, W2, a_src2, a_dst2, b2,
              g, mu, world=8):
    import ml_dtypes
    x16 = np.asarray(x, np.float32).astype(np.float16)
    N = x16.shape[0]
    NBLK = int(np.ceil(N / world / P))
    CAP = NBLK * P
    nbins = world * NBLK
    PB0 = min(NBLK, HALF // (world * P))
    PB1 = NBLK - PB0

    src = np.asarray(edge_index[0]).astype(np.int64)
    dst = np.asarray(edge_index[1]).astype(np.int64)

    # --- balanced global node -> (core, block, slot) assignment
    deg = np.bincount(dst, minlength=N)
    binid = _balance_bins(deg, nbins, P)
    order = np.lexsort((np.arange(N), binid))
    # local slot position within the core's shard
    lpos = np.empty(N, np.int64)
    nxt = np.arange(nbins, dtype=np.int64) * P
    for nd in order:
        b = binid[nd]
        lpos[nd] = nxt[b]
        nxt[b] += 1
    node_core = binid // NBLK
    node_blk = binid % NBLK
    lpos -= node_core * CAP              # position within own core [0, CAP)

    # global cc table position: AllGather piece-major
    # piece0 rows: [core, blocks 0:PB0]; piece1: [core, blocks PB0:NBLK]
    in_p1 = node_blk >= PB0
    gpos = np.where(
        ~in_p1,
        node_core * (PB0 * P) + lpos,
        world * PB0 * P + node_core * (PB1 * P) + (lpos - PB0 * P))

    # per-core list of node ids in shard slot order (-1 = empty slot)
    idxmaps = []
    for c in range(world):
        m = np.full(CAP, -1, np.int64)
        mask = node_core == c
        m[lpos[mask]] = np.nonzero(mask)[0]
        idxmaps.append(m)

    # --- edges grouped by (core, block) of dst
    ecore = node_core[dst]
    eblk = node_blk[dst]
    gkey = ecore * NBLK + eblk
    gorder = np.argsort(gkey, kind="stable")
    srcg, dstg, gkeyg = src[gorder], dst[gorder], gkey[gorder]
    starts = np.concatenate(
        [[0], np.cumsum(np.bincount(gkeyg, minlength=nbins))])

    ed = {}
    CPL = CPH = 1
    for c in range(world):
        for b in range(NBLK):
            gid = c * NBLK + b
            es = srcg[starts[gid]:starts[gid + 1]]
            eds = dstg[starts[gid]:starts[gid + 1]]
            dloc = (lpos[eds] - b * P).astype(np.int64)
            tl = gpos[es]
            lo = tl < world * PB0 * P
            ed[(c, b)] = (es, tl, lo, dloc)
            CPL = max(CPL, int(np.ceil(lo.sum() / P)))
            CPH = max(CPH, int(np.ceil((~lo).sum() / P)))

    cfg = CFG(N=N, W=world, NBLK=NBLK, CPL=CPL, CPH=CPH, idxmaps=idxmaps)
    CPB = cfg.CPB
    DCL, DCH, DCT = cfg.DCL, cfg.DCH, cfg.DCT
    ar128 = np.arange(P, dtype=np.int64)

    def build_core(c):
        isd = np.zeros((P, NBLK * CPB * 8), np.int16)
        sth = np.zeros((P, NBLK * CPB * P), ml_dtypes.bfloat16)
        salh = np.zeros((P, NBLK * CPB * P), ml_dtypes.bfloat16)
        srcs = np.zeros((NBLK, DCT * P), np.int64)    # dense-chunk x rows
        for b in range(NBLK):
            es, tl, lo, dloc = ed[(c, b)]
            fl = np.zeros(CPB * P, np.int64)      # slot -> table idx (pad 0)
            fd = np.full(CPB * P, -1, np.int64)   # slot -> dst_local (pad -1)
            fs = np.zeros(CPB * P, np.int64)      # slot -> src node id
            ilo = np.where(lo)[0]
            ihi = np.where(~lo)[0]
            fl[:len(ilo)] = tl[ilo]
            fd[:len(ilo)] = dloc[ilo]
            fs[:len(ilo)] = es[ilo]
            fl[CPL * P:CPL * P + len(ihi)] = tl[ihi] - world * PB0 * P
            fd[CPL * P:CPL * P + len(ihi)] = dloc[ihi]
            fs[CPL * P:CPL * P + len(ihi)] = es[ihi]
            cb8 = b * CPB * 8
            isd[:, cb8:cb8 + CPB * 8] = _wrap16(fl)
            # one-hots from fd [CPB, P]
            fdm = fd.reshape(CPB, P)
            oh = (fdm[:, :, None] == ar128)                 # [j, e, d]
            cbp = b * CPB * P
            sth[:, cbp:cbp + CPB * P] = \
                oh.transpose(2, 0, 1).reshape(P, CPB * P)   # st[d,(j,e)]
            salh[:, cbp:cbp + CPB * P] = \
                oh.transpose(1, 0, 2).reshape(P, CPB * P)   # sall[e,(j,d)]
            # dense chunk sources: lo [CPL-DCL:CPL], hi [CPB-DCH:CPB]
            fsm = fs.reshape(CPB, P)
            if DCL:
                srcs[b, 0:DCL * P] = fsm[CPL - DCL:CPL].ravel()
            if DCH:
                srcs[b, DCL * P:DCT * P] = fsm[CPB - DCH:CPB].ravel()
        xs = x16[srcs.ravel()]                    # [NBLK*DCT*P, IN]
        xsT = np.ascontiguousarray(
            xs.reshape(NBLK, DCT, P, 2, P).transpose(4, 0, 1, 3, 2))
        return isd, sth, salh, xsT

    # weights
    W1 = np.asarray(W1, np.float32)
    W2 = np.asarray(W2, np.float32)
    W1r = W1.reshape(H1, MD, IN)
    Ps1 = np.einsum("hdi,hd->ih", W1r, np.asarray(a_src1, np.float32))
    Pd1 = np.einsum("hdi,hd->ih", W1r, np.asarray(a_dst1, np.float32))
    W1aug = np.concatenate([W1.T, Ps1, Pd1], axis=1)
    Ps2 = W2.T @ np.asarray(a_src2, np.float32)[0][:, None]
    Pd2 = W2.T @ np.asarray(a_dst2, np.float32)[0][:, None]
    W2aug = np.concatenate([W2.T, Ps2, Pd2], axis=1)
    AUG1, AUG2 = IN + 4, IN + 2
    w1s = W1aug.reshape(2, P, AUG1).transpose(1, 0, 2).astype(np.float16)
    w2s = W2aug.reshape(2, P, AUG2).transpose(1, 0, 2).astype(np.float16)

    gm = np.asarray(g, np.float32)
    gsd = gm.reshape(2, P, KH * P).transpose(1, 0, 2).astype(np.float16)
    mu = np.asarray(mu, np.float32)
    mus = np.zeros((P, KH * KH), np.float16)
    onesd = np.zeros((P, KH * KH), np.float16)
    for k in range(KH):
        mus[:, k * KH + k] = mu[k, :]
        onesd[:, k * KH + k] = 1.0
    cmu = np.linalg.norm(mu, axis=1)[:, None].astype(np.float32)
    b1b = np.broadcast_to(np.asarray(b1, np.float32), (P, HID)).copy()
    b2b = np.broadcast_to(np.asarray(b2, np.float32), (P, OUT)).copy()
    ident = np.eye(P, dtype=np.float32)
    identb = np.eye(P, dtype=ml_dtypes.bfloat16)

    shared = dict(w1s=w1s, w2s=w2s, gs=gsd, mus=mus, onesd=onesd, cmu=cmu,
                  b1b=b1b, b2b=b2b, ident=ident, identb=identb)
    in_maps = []
    for c in range(world):
        m = idxmaps[c]
        own = np.where(m >= 0, m, 0)
        xo = x16[own]
        xo[m < 0] = 0
        xoT = np.ascontiguousarray(
            xo.reshape(NBLK, P, 2, P).transpose(3, 0, 2, 1))
        isd_c, st_c, sal_c, xsT_c = build_core(c)
        mm = dict(shared)
        mm.update(xoTi=xoT, xsTi=xsT_c, isd=isd_c, std=st_c, sald=sal_c)
        in_maps.append(mm)
    return cfg, in_maps


def assemble(cfg, outs):
    N = cfg.N
    full = np.zeros((N, KH), np.float32)
    for c in range(cfg.W):
        o = np.asarray(outs[c]["outT"])      # [KH, SHARD_CAP]
        m = cfg.idxmaps[c]
        valid = m >= 0
        full[m[valid], :] = o[:, valid].T
    return full


_CACHE = {}


def kernel(**inputs):
    world = 8
    cfg, in_maps = prep_host(world=world, **inputs)
    key = (cfg.N, cfg.W, cfg.CPL, cfg.CPH)
    if key not in _CACHE:
        _CACHE[key] = build_program(cfg)
    nc = _CACHE[key]

    from concourse.bass_utils import run_bass_kernel_spmd
    res = run_bass_kernel_spmd(nc, in_maps, core_ids=list(range(world)))
    return assemble(cfg, res.results)
